# revision 39
# baseline (speedup 1.0000x reference)
"""GraphSAGE (2-layer, mean-agg) Trainium2 Bass kernel, 8-core SPMD.

Design v3 (scatter-add L2):
- L1 dst-partitioned: each core owns 6250 dst nodes; fp16 x tables (xlo/xhi)
  replicated in DRAM; per-edge messages fetched with gpsimd dma_gather;
  segment-sum on the PE via per-rank selection matmuls.  S matrices are
  built on DVE with one tensor_scalar is_equal per rank against a fp16
  iota table; per-window dense compute yields p = h@w2_l and q = h@w2_r
  (40-dim each).  p rows are written to DRAM t_p (256B rows, node-id
  indexed).
- L2 src-partitioned via gpsimd dma_scatter_add: per-edge messages are the
  40-fp16 (80B) p rows, far below dma_gather's 256B-row minimum, so each
  edge costs ~7ns of descriptor time instead of ~22.8ns.  The core's own
  p rows are loaded into SBUF twice (dma_gather from t_p), sorted by the
  node's lo-half / hi-half out-degree; scatter "rounds" then push the k-th
  edge of every still-active node in one call, using count-sorted slot
  order so only trailing slots go inactive (the scatter API only skips
  trailing indices).  Nodes with degree > RCAP get ceil(deg/RCAP)
  replicated slots so RCAP rounds always suffice.
- int16 scatter indices cap the target space at 32768 rows, so the padded
  dst space (50176 rows) is split into lo/hi halves (25088 rows each, plus
  one trash row per half that absorbs cross-core padding slots).  lo/hi
  rounds are interleaved so the two write chains pipeline.
- The only collective is one ReduceScatter(add) of the [2,25088,40] fp16
  partial view (trash rows skipped via a strided AP) -> [6272,40] per core.
- Epilogue (recip-scale, +q(+b2 prefolded), log_softmax) runs once,
  batched fp16 [128,49,40], with Exp/Ln activation tables prewarmed.
"""
import numpy as np
import ml_dtypes

N = 50000
E = 800000
DIN, HID, OUT = 128, 512, 40
NCORES = 8
NLOC = N // NCORES            # 6250
P = 128
NW1 = 49                      # L1 windows (own nodes), node l = p*49 + k
NPAD1 = P * NW1               # 6272
XSPLIT = 32768                # x table split for int16 gather indices
L1_WCHUNK = 4                 # L1 windows per gather chunk
PTK = 16                      # L1 p-table write batch (windows)
PADVAL = 600.0                # drel pad: never matches iota (0..511)
RCAP_A = 12                   # scatter rounds cap per half, group A
RCAP_B = 8                    # scatter rounds cap per half, group B
KSPLIT = 25                   # L1 windows in group A (early-scatter group)
HALF = NPAD1 * (NCORES // 2)  # 25088 rows per half
TRASH = HALF                  # trash token (slot 196 -> acc group 98)

f16 = np.float16


def _wrap_call(flat_idx):
    """int16 wrapped layout for one gather call: slot i -> [i%16, i//16]."""
    n = len(flat_idx)
    w = flat_idx.astype(np.int16).reshape(n // 16, 16).T.copy()
    return np.tile(w, (8, 1))  # [128, n/16]


def _build_layer(per_core, nwin, wchunk, nbuckets):
    """Dense-rank schedule for L1 (see v2 docstring)."""
    srt = []
    for c in range(NCORES):
        row = []
        for b in range(nbuckets):
            idx, win, slot = per_core[c][b]
            o = np.argsort(win, kind="stable")
            row.append((idx[o], win[o], slot[o]))
        srt.append(row)

    chunks = []
    idx_cols = [[] for _ in range(NCORES)]
    drel_cols = [[] for _ in range(NCORES)]
    rank_id = 0
    cum16 = 0
    if isinstance(wchunk, list):
        bounds = wchunk
    else:
        bounds = list(range(0, nwin, wchunk)) + [nwin]
    for ci, (w0, w1) in enumerate(zip(bounds[:-1], bounds[1:])):
        calls = []
        ranks_meta = []
        win_ops = {w: [] for w in range(w0, w1)}
        msg_off = 0
        for b in range(nbuckets):
            sel = []
            cnts = []
            for c in range(NCORES):
                idx, win, slot = srt[c][b]
                lo = np.searchsorted(win, w0, "left")
                hi = np.searchsorted(win, w1, "left")
                sel.append((idx[lo:hi], win[lo:hi], slot[lo:hi]))
                cnts.append(hi - lo)
            nr = (max(cnts) + P - 1) // P
            if nr == 0:
                continue
            nidx = nr * P
            # trimmed transfer count (x16); msg buffers are zeroed once up
            # front so un-transferred tail slots stay finite.
            used = -(-max(cnts) // 16) * 16
            cover = np.full((nr, 2), -1, np.int64)
            for c in range(NCORES):
                idx, win, slot = sel[c]
                flat = np.zeros(used, np.int64)
                drl = np.full(nidx, PADVAL, np.float64)
                ne = len(idx)
                flat[:ne] = idx
                drl[:ne] = slot + 128 * (win & 3)
                idx_cols[c].append(_wrap_call(flat))
                drel_cols[c].append(drl.reshape(nr, P).T.astype(np.float32))
                for r in range(nr):
                    a, z = r * P, min((r + 1) * P, ne)
                    if a >= ne:
                        break
                    wmin, wmax = win[a], win[z - 1]
                    if cover[r, 0] < 0:
                        cover[r] = (wmin, wmax)
                    else:
                        cover[r, 0] = min(cover[r, 0], wmin)
                        cover[r, 1] = max(cover[r, 1], wmax)
            calls.append((b, nidx, used, cum16))
            cum16 += used // 16
            for r in range(nr):
                wmin, wmax = cover[r]
                if wmin < 0:
                    continue
                span = int(wmax - wmin + 1)
                assert span <= 4, f"rank spans {span} windows"
                rid = rank_id + r
                ranks_meta.append((rid, msg_off + r, int(wmin) & 3, span))
                for w in range(int(wmin), int(wmax) + 1):
                    win_ops[w].append((rid, w - int(wmin)))
            rank_id += nr
            msg_off += nr
        chunks.append({
            "calls": calls,
            "nranks": msg_off,
            "ranks": ranks_meta,
            "windows": [(w, win_ops[w]) for w in range(w0, w1)],
        })
    idx_arr = [np.concatenate(idx_cols[c], axis=1) if idx_cols[c]
               else np.zeros((P, 0), np.int16) for c in range(NCORES)]
    drel_arr = [np.concatenate(drel_cols[c], axis=1) if drel_cols[c]
                else np.zeros((P, 0), np.float32) for c in range(NCORES)]
    return idx_arr, drel_arr, chunks, rank_id


def _enc_idx(d):
    """Scatter token id for global dst d: partition/slot encoding whose
    parity-split SBUF accumulators reassemble densely into row 6272*o+dl."""
    o = d // NLOC
    dl = d % NLOC
    prel = dl // 196
    q = dl % 196
    t = q // 98
    g = q % 98
    s = 2 * g + t
    return (o // 4), 128 * s + 32 * (o % 4) + prel  # (half, idx)


def _rowp(l):
    """win-major t_p row of local node l (computed in window l%49, slot l//49)."""
    return (l % NW1) * P + l // NW1


def _build_scatter(src, dst):
    """L2 scatter-add schedule: src-partitioned, lo/hi dst halves, and A/B
    source-window groups (A = L1 windows [0, KSPLIT) so its scatters can
    start mid-L1).

    Returns per-core gather/scatter idx tables plus the shared round
    structure: rounds = [(grp, phase, m_k, col16, ncol16)], Spad[grp][ph].
    """
    RCAP = [RCAP_A, RCAP_B]
    percore = []  # [c][g][ph] = (gidx_sorted(rows), counts_sorted, Amat)
    for c in range(NCORES):
        m = (src >= c * NLOC) & (src < (c + 1) * NLOC)
        s_all = src[m] - c * NLOC
        d = dst[m]
        half_all, idx_all = _enc_idx(d)
        grp_all = (s_all % NW1 >= KSPLIT).astype(np.int64)
        gdata = []
        for g in (0, 1):
            ph_data = []
            for ph in (0, 1):
                sel = (half_all == ph) & (grp_all == g)
                ss = s_all[sel]
                rr = idx_all[sel]
                o = np.argsort(ss, kind="stable")
                ss, rr = ss[o], rr[o]
                deg = np.bincount(ss, minlength=NLOC)
                nz = np.nonzero(deg)[0]
                gidx, lists = [], []
                pos = 0
                for i in nz:
                    dg = int(deg[i])
                    ci = -(-dg // RCAP[g])
                    lst = rr[pos:pos + dg]
                    for j in range(ci):
                        gidx.append(_rowp(int(i)))
                        lists.append(lst[j::ci])
                    pos += dg
                counts = np.fromiter((len(x) for x in lists), np.int64,
                                     len(lists))
                order = np.argsort(-counts, kind="stable")
                gidx = np.asarray(gidx, np.int64)[order]
                counts = counts[order]
                A = np.full((len(lists), RCAP[g]), TRASH, np.int64)
                for q, oi in enumerate(order):
                    A[q, :counts[q]] = lists[oi]
                ph_data.append((gidx, counts, A))
            gdata.append(ph_data)
        percore.append(gdata)

    Spad = [[0, 0], [0, 0]]
    for g in (0, 1):
        for ph in (0, 1):
            smax = max(len(percore[c][g][ph][0]) for c in range(NCORES))
            Spad[g][ph] = -(-smax // P) * P

    rounds = []
    col16 = 0
    for g in (0, 1):
        for k in range(RCAP[g]):
            for ph in (0, 1):
                m_k = max(int((percore[c][g][ph][1] > k).sum())
                          for c in range(NCORES))
                if m_k == 0:
                    continue
                ncol = -(-m_k // 16)
                rounds.append((g, k, ph, m_k, col16, ncol))
                col16 += ncol

    zero_row = _rowp(NPAD1 - 1)  # padded node -> all-zero p row
    g_arr = [[[None, None], [None, None]] for _ in range(NCORES)]
    sc_arr = []
    for c in range(NCORES):
        for g in (0, 1):
            for ph in (0, 1):
                gidx = percore[c][g][ph][0]
                gp = np.full(Spad[g][ph], zero_row, np.int64)
                gp[:len(gidx)] = gidx
                g_arr[c][g][ph] = _wrap_call(gp)
        cols = []
        for (g, k, ph, m_k, c16, ncol) in rounds:
            A = percore[c][g][ph][2]
            arr = np.full(ncol * 16, TRASH, np.int64)
            take = min(m_k, A.shape[0])
            arr[:take] = A[:take, k]
            arr[m_k:] = -1  # beyond num_idxs: trailing pad
            cols.append(_wrap_call(arr))
        sc_arr.append(np.concatenate(cols, axis=1) if cols
                      else np.zeros((P, 0), np.int16))
    return g_arr, sc_arr, rounds, Spad


def _build_schedule(edge_index):
    src = np.asarray(edge_index[0], dtype=np.int64)
    dst = np.asarray(edge_index[1], dtype=np.int64)
    deg = np.bincount(dst, minlength=N).astype(np.float32)
    recip = 1.0 / np.maximum(deg, 1.0)

    # L1: dst-partitioned; window/slot from local node l = p*49 + k
    l1 = []
    for c in range(NCORES):
        m = (dst >= c * NLOC) & (dst < (c + 1) * NLOC)
        s, d = src[m], dst[m] - c * NLOC
        win = d % NW1
        slot = d // NW1
        blo = s < XSPLIT
        l1.append([
            (s[blo], win[blo], slot[blo]),
            (s[~blo] - XSPLIT, win[~blo], slot[~blo]),
        ])
    i1, d1, chunks1, R1 = _build_layer(
        l1, NW1, list(range(0, 45, L1_WCHUNK)) + [46, 48, NW1], 2)

    g_arr, sc_arr, rounds, Spad = _build_scatter(src, dst)
    return i1, d1, chunks1, R1, g_arr, sc_arr, rounds, Spad, recip


def kernel(x, edge_index, w1_l, b1, w1_r, w2_l, b2, w2_r):
    import concourse.bacc as bacc
    import concourse.mybir as mybir
    import concourse.tile as tile
    from concourse.bass_utils import run_bass_kernel_spmd
    from concourse.library_config import mlp
    from concourse.masks import make_identity

    x = np.asarray(x, np.float32)
    (i1, d1, chunks1, R1, g_arr, sc_arr, rounds, Spad,
     recip) = _build_schedule(np.asarray(edge_index))
    CRMAX = max(ch["nranks"] for ch in chunks1)

    xlo = x[:XSPLIT].astype(f16)
    xhi = x[XSPLIT:].astype(f16)
    iota_np = np.tile((np.arange(1024) % 512).astype(np.float32)[None, :],
                      (P, 1)).astype(f16)

    T16_1 = i1[0].shape[1]
    T16_S = sc_arr[0].shape[1]

    nc = bacc.Bacc("TRN2", dynamic_dma_scratch_size=49152)
    dt = mybir.dt
    t_xlo = nc.declare_dram_parameter("xlo", [XSPLIT, DIN], dt.float16, isOutput=False)
    t_xhi = nc.declare_dram_parameter("xhi", [N - XSPLIT, DIN], dt.float16, isOutput=False)
    t_i1 = nc.declare_dram_parameter("i1", [P, T16_1], dt.int16, isOutput=False)
    t_d1 = nc.declare_dram_parameter("d1", [P, R1], dt.float32, isOutput=False)
    t_g = [[nc.declare_dram_parameter(f"g{g}{ph}", [P, Spad[g][ph] // 16],
                                      dt.int16, isOutput=False)
            for ph in (0, 1)] for g in (0, 1)]
    t_sc = nc.declare_dram_parameter("sc", [P, T16_S], dt.int16, isOutput=False)
    t_xoT = nc.declare_dram_parameter("xoT", [DIN, NW1, P], dt.float16, isOutput=False)
    t_w1l = nc.declare_dram_parameter("w1l", [DIN, HID], dt.float16, isOutput=False)
    t_w1r = nc.declare_dram_parameter("w1r", [DIN, HID], dt.float16, isOutput=False)
    t_w2l = nc.declare_dram_parameter("w2l", [P, HID // P, OUT], dt.float16, isOutput=False)
    t_w2r = nc.declare_dram_parameter("w2r", [P, HID // P, OUT], dt.float16, isOutput=False)
    t_b1 = nc.declare_dram_parameter("b1", [P, HID // P], dt.float32, isOutput=False)
    t_b2 = nc.declare_dram_parameter("b2r", [P, OUT], dt.float32, isOutput=False)
    t_rc = nc.declare_dram_parameter("rc", [P, NW1], dt.float32, isOutput=False)
    t_rc40 = nc.declare_dram_parameter("rc40", [P, NW1, OUT], dt.float16, isOutput=False)
    t_iota = nc.declare_dram_parameter("iota", [P, 1024], dt.float16, isOutput=False)
    t_out = nc.declare_dram_parameter("out", [P, NW1, OUT], dt.float16, isOutput=True)

    t_p = nc.dram_tensor("ptab", [NW1 * P, DIN], dt.float16)  # row w*128+slot
    t_cont = nc.dram_tensor("cont", [NCORES * NPAD1, OUT], dt.float8e4)
    t_rs = nc.dram_tensor("rsout", [NPAD1, OUT], dt.float8e4)

    AluOp = mybir.AluOpType
    AF = mybir.ActivationFunctionType

    MAXSPAD = max(max(row) for row in Spad)
    with tile.TileContext(nc) as tc:
        with tc.tile_pool(name="const", bufs=1) as cpool, \
             tc.tile_pool(name="scat", bufs=1) as zpool:
            nc.gpsimd.load_library(mlp)
            ident = cpool.tile([P, P], dt.float16)
            make_identity(nc, ident[:])
            # i1 first so gather-0 starts early; big/late tables (xoT, sc,
            # gl, gh, rc40) queue after it.
            i1_t = cpool.tile([P, T16_1], dt.int16)
            nc.sync.dma_start(i1_t[:], t_i1[:])
            d1_t = cpool.tile([P, R1], dt.float32)
            nc.sync.dma_start(d1_t[:], t_d1[:])
            iota_t = cpool.tile([P, 1024], dt.float16)
            nc.sync.dma_start(iota_t[:], t_iota[:])
            w1l_t = cpool.tile([DIN, HID], dt.float16)
            nc.sync.dma_start(w1l_t[:], t_w1l[:])
            w1r_t = cpool.tile([DIN, HID], dt.float16)
            nc.sync.dma_start(w1r_t[:], t_w1r[:])
            w2l_t = cpool.tile([P, HID // P, OUT], dt.float16)
            nc.sync.dma_start(w2l_t[:], t_w2l[:])
            w2r_t = cpool.tile([P, HID // P, OUT], dt.float16)
            nc.sync.dma_start(w2r_t[:], t_w2r[:])
            b1_t = cpool.tile([P, HID // P], dt.float32)
            nc.sync.dma_start(b1_t[:], t_b1[:])
            b2_t = cpool.tile([P, OUT], dt.float32)
            nc.sync.dma_start(b2_t[:], t_b2[:])
            rc_t = cpool.tile([P, NW1], dt.float32)
            nc.sync.dma_start(rc_t[:], t_rc[:])
            # late tables: tiles allocated here, loads issued from Pool after
            # the first gather preps so they don't hog the DMA queue early
            xoT_t = cpool.tile([DIN, NW1, P], dt.float16)
            sc_t = cpool.tile([P, T16_S], dt.int16)
            g_t = [[cpool.tile([P, Spad[g][ph] // 16], dt.int16,
                               tag=f"g{g}{ph}", name=f"g{g}{ph}t")
                    for ph in (0, 1)] for g in (0, 1)]
            rc40_t = cpool.tile([P, NW1, OUT], dt.float16)
            qbuf = cpool.tile([P, NW1, OUT], dt.float16)
            warm = cpool.tile([P, 2], dt.float32)
            nc.scalar.activation(warm[:, 0:1], b2_t[:, 0:1], AF.Exp)
            nc.scalar.activation(warm[:, 1:2], b2_t[:, 1:2], AF.Ln)
            # per-window p staging (cols OUT: stay zero from the memset)
            pws = []
            for i in range(3):
                pw = cpool.tile([P, DIN], dt.float16, tag=f"pw{i}")
                nc.vector.memset(pw[:], 0.0)
                pws.append(pw)
            # scatter accumulators [half*2+parity] and per-group p tiles
            acc_all = zpool.tile([P, 4, 99, OUT], dt.float16)
            nc.vector.memset(acc_all[:], 0.0)
            accs = [[acc_all[:, 2 * h + t] for t in (0, 1)] for h in (0, 1)]
            p_t = [[zpool.tile([P, Spad[g][ph] // P, OUT], dt.float16,
                               tag=f"p{g}{ph}", name=f"p{g}{ph}t")
                    for ph in (0, 1)] for g in (0, 1)]
            praw = zpool.tile([P, MAXSPAD // P, DIN], dt.float16)

            # ---------------- Layer 1 ----------------
            with tc.tile_pool(name="msg", bufs=3) as mpool, \
                 tc.tile_pool(name="sm", bufs=16) as spool, \
                 tc.tile_pool(name="work", bufs=3) as wpool, \
                 tc.tile_pool(name="psumA", bufs=2, space="PSUM") as ppool, \
                 tc.tile_pool(name="psumB", bufs=2, space="PSUM") as ppoolb, \
                 tc.tile_pool(name="psumC", bufs=1, space="PSUM") as ppoolc:
                for _mb in range(3):
                    mz = mpool.tile([P, CRMAX, DIN], dt.float16, tag="msg")
                    nc.vector.memset(mz[:], 0.0)
                wcount = 0           # windows completed (for pw rotation)
                for ci, ch in enumerate(chunks1):
                    cr = ch["nranks"]
                    if cr == 0:
                        continue
                    msg = mpool.tile([P, CRMAX, DIN], dt.float16, tag="msg")
                    off = 0
                    for b, nidx, used, cum16 in ch["calls"]:
                        tbl = t_xlo[:] if b == 0 else t_xhi[:]
                        nc.gpsimd.dma_gather(
                            msg[:, off:off + (-(-used // P)), :], tbl,
                            i1_t[:, cum16:cum16 + used // 16],
                            used, used, DIN, single_packet=False)
                        off += nidx // P
                    if ci == 0:
                        nc.gpsimd.dma_start(xoT_t[:], t_xoT[:])
                    elif ci == 1:
                        nc.gpsimd.dma_start(sc_t[:], t_sc[:])
                    elif ci == 2:
                        for g in (0, 1):
                            for ph2 in (0, 1):
                                nc.gpsimd.dma_start(g_t[g][ph2][:],
                                                    t_g[g][ph2][:])
                        nc.gpsimd.dma_start(rc40_t[:], t_rc40[:])
                    elif ci == 8:
                        # group-A p tables: rows [0, KSPLIT*128) are final
                        for ph2 in (0, 1):
                            sp = Spad[0][ph2]
                            nc.gpsimd.dma_gather(
                                praw[:, :sp // P, :], t_p[0:KSPLIT * P, :],
                                g_t[0][ph2][:], sp, sp, DIN,
                                single_packet=False)
                            nc.scalar.activation(p_t[0][ph2][:],
                                                 praw[:, :sp // P, :OUT],
                                                 AF.Copy)
                    rank_info = {rid: (lr, q0, span) for rid, lr, q0, span in ch["ranks"]}
                    S_tiles = {}
                    for w, ops in ch["windows"]:
                        for rid, blk in ops:
                            if rid not in S_tiles:
                                lr, q0, span = rank_info[rid]
                                S = spool.tile([P, 512], dt.float16, tag="S1")
                                nc.vector.tensor_scalar(
                                    out=S[:, :span * P],
                                    in0=iota_t[:, q0 * P:(q0 + span) * P],
                                    scalar1=d1_t[:, rid:rid + 1], scalar2=None,
                                    op0=AluOp.is_equal)
                                S_tiles[rid] = S
                        pagg = ppool.tile([P, P], dt.float32, tag="pagg")
                        if not ops:
                            nc.vector.memset(pagg[:], 0.0)
                        for j, (rid, blk) in enumerate(ops):
                            lr = rank_info[rid][0]
                            nc.tensor.matmul(
                                pagg[:], lhsT=S_tiles[rid][:, blk * P:(blk + 1) * P],
                                rhs=msg[:, lr, :],
                                start=(j == 0), stop=(j == len(ops) - 1))
                        am = wpool.tile([P, P], dt.float16, tag="am")
                        nc.scalar.activation(am[:], pagg[:], AF.Copy,
                                             scale=rc_t[:, w:w + 1])
                        pamT = ppoolc.tile([P, P], dt.float16, tag="pamT")
                        nc.tensor.transpose(out=pamT[:], in_=am[:], identity=ident[:])
                        amT = wpool.tile([P, P], dt.float16, tag="amT")
                        nc.scalar.activation(amT[:], pamT[:], AF.Copy)
                        pq = ppoolc.tile([P, OUT], dt.float32, tag="pq")
                        qq = ppoolc.tile([P, OUT], dt.float32, tag="qq")
                        for bjj in range(HID // P):
                            ph_ = ppoolb.tile([P, P], dt.float32, tag="ph")
                            nc.tensor.matmul(ph_[:], lhsT=w1l_t[:, bjj * P:(bjj + 1) * P],
                                             rhs=amT[:], start=True, stop=False)
                            nc.tensor.matmul(ph_[:], lhsT=w1r_t[:, bjj * P:(bjj + 1) * P],
                                             rhs=xoT_t[:, w, :], start=False, stop=True)
                            hT = wpool.tile([P, P], dt.float16, tag="hT")
                            nc.scalar.activation(hT[:], ph_[:], AF.Relu,
                                                 bias=b1_t[:, bjj:bjj + 1])
                            nc.tensor.matmul(pq[:], lhsT=hT[:], rhs=w2l_t[:, bjj, :],
                                             start=(bjj == 0), stop=(bjj == 3))
                            nc.tensor.matmul(qq[:], lhsT=hT[:], rhs=w2r_t[:, bjj, :],
                                             start=(bjj == 0), stop=(bjj == 3))
                        nc.scalar.activation(qbuf[:, w, :], qq[:], AF.Copy)
                        pw = pws[wcount % 3]
                        wcount += 1
                        nc.scalar.activation(pw[:, :OUT], pq[:], AF.Copy)
                        nc.sync.dma_start(t_p[w * P:(w + 1) * P, :], pw[:])

            # fold b2 into qbuf early (overlaps L2 phase / RS wait)
            nc.vector.tensor_tensor(
                out=qbuf[:], in0=qbuf[:],
                in1=b2_t[:, None, :].to_broadcast([P, NW1, OUT]), op=AluOp.add)

            # ---------------- Layer 2 tail: A rounds, B build+rounds ---------
            with tc.tile_pool(name="epi", bufs=1) as epool:
                b_built = False
                for g, k, ph, m_k, c16, ncol in rounds:
                    if g == 1 and not b_built:
                        b_built = True
                        # group-B p tables (need the full t_p)
                        for ph2 in (0, 1):
                            sp = Spad[1][ph2]
                            nc.gpsimd.dma_gather(
                                praw[:, :sp // P, :], t_p[:],
                                g_t[1][ph2][:], sp, sp, DIN,
                                single_packet=False)
                            nc.scalar.activation(p_t[1][ph2][:],
                                                 praw[:, :sp // P, :OUT],
                                                 AF.Copy)
                    nr = -(-m_k // P)
                    nc.gpsimd.dma_scatter_add(
                        accs[ph][0][:], p_t[g][ph][:, :nr, :],
                        sc_t[:, c16:c16 + ncol],
                        m_k, m_k, OUT,
                        single_packet=False,
                        sbuf_tokens_per_rank=P,
                        parity_reg=0,
                        out_ap_other=accs[ph][1][:])

                # dense write-out: row 25088*h + 196*p + 98*t + g == 6272*o + dl
                acc8 = epool.tile([P, 4, 98, OUT], dt.float8e4, tag="acc8")
                nc.scalar.activation(acc8[:], acc_all[:, :, :98, :], AF.Copy)
                tc_v = t_cont[:].rearrange("(h p q) j -> h p q j", h=2, p=P)
                for h in (0, 1):
                    for t in (0, 1):
                        nc.sync.dma_start(tc_v[h][:, 98 * t:98 * (t + 1), :],
                                          acc8[:, 2 * h + t])

                # ---------------- ReduceScatter + epilogue ----------------
                nc.gpsimd.collective_compute(
                    "ReduceScatter", AluOp.add, replica_groups=[list(range(NCORES))],
                    ins=[t_cont[:]], outs=[t_rs[:]])

                rsb = epool.tile([P, NW1, OUT], dt.float8e4, tag="rsb")
                nc.sync.dma_start(
                    rsb[:], t_rs[:].rearrange("(p k) j -> p k j", p=P))
                HALVES = [(0, 25), (25, NW1)]
                zts, ezs, ssums, lsums = [], [], [], []
                for h, (k0, k1) in enumerate(HALVES):
                    KH = k1 - k0
                    zt = epool.tile([P, KH, OUT], dt.float16, tag=f"zt{h}")
                    nc.vector.tensor_tensor(out=zt[:], in0=rsb[:, k0:k1, :],
                                            in1=rc40_t[:, k0:k1, :], op=AluOp.mult)
                    nc.vector.tensor_tensor(out=zt[:], in0=zt[:],
                                            in1=qbuf[:, k0:k1, :], op=AluOp.add)
                    zts.append(zt)
                    ez = epool.tile([P, KH, OUT], dt.float32, tag=f"ez{h}")
                    nc.scalar.activation(ez[:], zt[:], AF.Exp)
                    ezs.append(ez)
                for h, (k0, k1) in enumerate(HALVES):
                    KH = k1 - k0
                    ssum = epool.tile([P, KH, 1], dt.float32, tag=f"ssum{h}")
                    nc.vector.tensor_reduce(ssum[:], ezs[h][:],
                                            axis=mybir.AxisListType.X, op=AluOp.add)
                    ssums.append(ssum)
                for h, (k0, k1) in enumerate(HALVES):
                    KH = k1 - k0
                    lsum = epool.tile([P, KH, 1], dt.float32, tag=f"lsum{h}")
                    nc.scalar.activation(lsum[:], ssums[h][:], AF.Ln)
                    lsums.append(lsum)
                for h, (k0, k1) in enumerate(HALVES):
                    KH = k1 - k0
                    ot = epool.tile([P, KH, OUT], dt.float16, tag=f"ot{h}")
                    nc.vector.tensor_tensor(
                        out=ot[:], in0=zts[h][:],
                        in1=lsums[h][:].to_broadcast([P, KH, OUT]), op=AluOp.subtract)
                    nc.sync.dma_start(t_out[:, k0:k1, :], ot[:])

    nc.compile()

    b2b_np = np.tile(np.asarray(b2, np.float32)[None, :], (P, 1))
    in_maps = []
    for c in range(NCORES):
        xl = np.zeros((NPAD1, DIN), np.float32)
        xl[:NLOC] = x[c * NLOC:(c + 1) * NLOC]
        xoT = np.ascontiguousarray(
            xl.reshape(P, NW1, DIN).transpose(2, 1, 0)).astype(f16)
        rcf = np.ones(NPAD1, np.float32)
        rcf[:NLOC] = recip[c * NLOC:(c + 1) * NLOC]
        rcc = rcf.reshape(P, NW1).copy()
        in_maps.append({
            "xlo": xlo, "xhi": xhi,
            "i1": i1[c], "d1": d1[c],
            "g00": g_arr[c][0][0], "g01": g_arr[c][0][1],
            "g10": g_arr[c][1][0], "g11": g_arr[c][1][1],
            "sc": sc_arr[c],
            "xoT": xoT, "rc": rcc, "iota": iota_np,
            "rc40": np.ascontiguousarray(np.broadcast_to(rcc[:, :, None], (P, NW1, OUT))).astype(f16),
            "w1l": np.asarray(w1_l).astype(f16),
            "w1r": np.asarray(w1_r).astype(f16),
            "w2l": np.ascontiguousarray(
                np.asarray(w2_l).astype(f16).reshape(HID // P, P, OUT)
                .transpose(1, 0, 2)),
            "w2r": np.ascontiguousarray(
                np.asarray(w2_r).astype(f16).reshape(HID // P, P, OUT)
                .transpose(1, 0, 2)),
            "b1": np.asarray(b1, np.float32).reshape(HID // P, P).T.copy(),
            "b2r": b2b_np,
        })
    res = run_bass_kernel_spmd(nc, in_maps, list(range(NCORES)))
    out = np.concatenate(
        [res.results[c]["out"].reshape(NPAD1, OUT)[:NLOC] for c in range(NCORES)],
        axis=0)
    kernel.last_results = res
    kernel.last_nc = nc
    return out.astype(np.float32)


# revision 43
# speedup vs baseline: 1.1329x; 1.1329x over previous
"""GraphSAGE (2-layer, mean-agg) Trainium2 Bass kernel, 8-core SPMD.

Design v3 (scatter-add L2):
- L1 dst-partitioned: each core owns 6250 dst nodes; fp16 x tables (xlo/xhi)
  replicated in DRAM; per-edge messages fetched with gpsimd dma_gather;
  segment-sum on the PE via per-rank selection matmuls.  S matrices are
  built on DVE with one tensor_scalar is_equal per rank against a fp16
  iota table; per-window dense compute yields p = h@w2_l and q = h@w2_r
  (40-dim each).  p rows are written to DRAM t_p (256B rows, node-id
  indexed).
- L2 src-partitioned via gpsimd dma_scatter_add: per-edge messages are the
  40-fp16 (80B) p rows, far below dma_gather's 256B-row minimum, so each
  edge costs ~7ns of descriptor time instead of ~22.8ns.  The core's own
  p rows are loaded into SBUF twice (dma_gather from t_p), sorted by the
  node's lo-half / hi-half out-degree; scatter "rounds" then push the k-th
  edge of every still-active node in one call, using count-sorted slot
  order so only trailing slots go inactive (the scatter API only skips
  trailing indices).  Nodes with degree > RCAP get ceil(deg/RCAP)
  replicated slots so RCAP rounds always suffice.
- int16 scatter indices cap the target space at 32768 rows, so the padded
  dst space (50176 rows) is split into lo/hi halves (25088 rows each, plus
  one trash row per half that absorbs cross-core padding slots).  lo/hi
  rounds are interleaved so the two write chains pipeline.
- The only collective is one ReduceScatter(add) of the [2,25088,40] fp16
  partial view (trash rows skipped via a strided AP) -> [6272,40] per core.
- Epilogue (recip-scale, +q(+b2 prefolded), log_softmax) runs once,
  batched fp16 [128,49,40], with Exp/Ln activation tables prewarmed.
"""
import numpy as np
import ml_dtypes

N = 50000
E = 800000
DIN, HID, OUT = 128, 512, 40
NCORES = 8
NLOC = N // NCORES            # 6250
P = 128
NW1 = 49                      # L1 windows (own nodes), node l = p*49 + k
NPAD1 = P * NW1               # 6272
XSPLIT = 32768                # x table split for int16 gather indices
L1_WCHUNK = 4                 # L1 windows per gather chunk
PTK = 16                      # L1 p-table write batch (windows)
PADVAL = 600.0                # drel pad: never matches iota (0..511)
RCAP = 12                     # scatter rounds cap per half
HALF = NPAD1 * (NCORES // 2)  # 25088 rows per half
TRASH = HALF                  # trash row index within each half view
NPZ_ROWS = 2 * (HALF + 1)     # 50178 rows in the scatter target

f16 = np.float16


def _wrap_call(flat_idx):
    """int16 wrapped layout for one gather call: slot i -> [i%16, i//16]."""
    n = len(flat_idx)
    w = flat_idx.astype(np.int16).reshape(n // 16, 16).T.copy()
    return np.tile(w, (8, 1))  # [128, n/16]


def _build_layer(per_core, nwin, wchunk, nbuckets):
    """Dense-rank schedule for L1 (see v2 docstring)."""
    srt = []
    for c in range(NCORES):
        row = []
        for b in range(nbuckets):
            idx, win, slot = per_core[c][b]
            o = np.argsort(win, kind="stable")
            row.append((idx[o], win[o], slot[o]))
        srt.append(row)

    chunks = []
    idx_cols = [[] for _ in range(NCORES)]
    drel_cols = [[] for _ in range(NCORES)]
    rank_id = 0
    cum16 = 0
    if isinstance(wchunk, list):
        bounds = wchunk
    else:
        bounds = list(range(0, nwin, wchunk)) + [nwin]
    for ci, (w0, w1) in enumerate(zip(bounds[:-1], bounds[1:])):
        calls = []
        ranks_meta = []
        win_ops = {w: [] for w in range(w0, w1)}
        msg_off = 0
        for b in range(nbuckets):
            sel = []
            cnts = []
            for c in range(NCORES):
                idx, win, slot = srt[c][b]
                lo = np.searchsorted(win, w0, "left")
                hi = np.searchsorted(win, w1, "left")
                sel.append((idx[lo:hi], win[lo:hi], slot[lo:hi]))
                cnts.append(hi - lo)
            nr = (max(cnts) + P - 1) // P
            if nr == 0:
                continue
            nidx = nr * P
            # trimmed transfer count (x16); msg buffers are zeroed once up
            # front so un-transferred tail slots stay finite.
            used = -(-max(cnts) // 16) * 16
            cover = np.full((nr, 2), -1, np.int64)
            for c in range(NCORES):
                idx, win, slot = sel[c]
                flat = np.zeros(used, np.int64)
                drl = np.full(nidx, PADVAL, np.float64)
                ne = len(idx)
                flat[:ne] = idx
                drl[:ne] = slot + 128 * (win & 3)
                idx_cols[c].append(_wrap_call(flat))
                drel_cols[c].append(drl.reshape(nr, P).T.astype(np.float32))
                for r in range(nr):
                    a, z = r * P, min((r + 1) * P, ne)
                    if a >= ne:
                        break
                    wmin, wmax = win[a], win[z - 1]
                    if cover[r, 0] < 0:
                        cover[r] = (wmin, wmax)
                    else:
                        cover[r, 0] = min(cover[r, 0], wmin)
                        cover[r, 1] = max(cover[r, 1], wmax)
            calls.append((b, nidx, used, cum16))
            cum16 += used // 16
            for r in range(nr):
                wmin, wmax = cover[r]
                if wmin < 0:
                    continue
                span = int(wmax - wmin + 1)
                assert span <= 4, f"rank spans {span} windows"
                rid = rank_id + r
                ranks_meta.append((rid, msg_off + r, int(wmin) & 3, span))
                for w in range(int(wmin), int(wmax) + 1):
                    win_ops[w].append((rid, w - int(wmin)))
            rank_id += nr
            msg_off += nr
        chunks.append({
            "calls": calls,
            "nranks": msg_off,
            "ranks": ranks_meta,
            "windows": [(w, win_ops[w]) for w in range(w0, w1)],
        })
    idx_arr = [np.concatenate(idx_cols[c], axis=1) if idx_cols[c]
               else np.zeros((P, 0), np.int16) for c in range(NCORES)]
    drel_arr = [np.concatenate(drel_cols[c], axis=1) if drel_cols[c]
                else np.zeros((P, 0), np.float32) for c in range(NCORES)]
    return idx_arr, drel_arr, chunks, rank_id


def _enc_idx(d):
    """Scatter token id for global dst d: partition/slot encoding whose
    parity-split SBUF accumulators reassemble densely into row 6272*o+dl."""
    o = d // NLOC
    dl = d % NLOC
    prel = dl // 196
    q = dl % 196
    t = q // 98
    g = q % 98
    s = 2 * g + t
    return (o // 4), 128 * s + 32 * (o % 4) + prel  # (half, idx)


def _build_scatter(src, dst):
    """L2 scatter-add schedule (src-partitioned, lo/hi dst halves).

    Returns per-core gather/scatter idx tables plus the shared round
    structure: rounds = [(phase, m_k, col16, ncol16)], Spad[2].
    """
    percore = []  # [c][ph] = (gidx_sorted, counts_sorted)
    Amats = []    # [c][ph] = slot-major [S, RCAP] edge-target idxs (TRASH pad)
    for c in range(NCORES):
        m = (src >= c * NLOC) & (src < (c + 1) * NLOC)
        s_all = src[m] - c * NLOC
        d = dst[m]
        half_all, idx_all = _enc_idx(d)
        ph_data = []
        ph_A = []
        for ph in (0, 1):
            sel = half_all == ph
            ss = s_all[sel]
            rr = idx_all[sel]
            o = np.argsort(ss, kind="stable")
            ss, rr = ss[o], rr[o]
            deg = np.bincount(ss, minlength=NLOC)
            nz = np.nonzero(deg)[0]
            gidx, lists = [], []
            pos = 0
            for i in nz:
                dg = int(deg[i])
                ci = -(-dg // RCAP)
                lst = rr[pos:pos + dg]
                for j in range(ci):
                    gidx.append(i)
                    lists.append(lst[j::ci])
                pos += dg
            counts = np.fromiter((len(x) for x in lists), np.int64, len(lists))
            order = np.argsort(-counts, kind="stable")
            gidx = np.asarray(gidx, np.int64)[order]
            counts = counts[order]
            A = np.full((len(lists), RCAP), TRASH, np.int64)
            for q, oi in enumerate(order):
                A[q, :counts[q]] = lists[oi]
            ph_data.append((gidx, counts))
            ph_A.append(A)
        percore.append(ph_data)
        Amats.append(ph_A)

    Spad = []
    for ph in (0, 1):
        smax = max(len(percore[c][ph][0]) for c in range(NCORES))
        Spad.append(-(-smax // P) * P)

    rounds = []
    col16 = 0
    for k in range(RCAP):
        for ph in (0, 1):
            m_k = max(int((percore[c][ph][1] > k).sum()) for c in range(NCORES))
            if m_k == 0:
                continue
            ncol = -(-m_k // 16)
            rounds.append((ph, m_k, col16, ncol))
            col16 += ncol

    g_arr = [[None, None] for _ in range(NCORES)]
    sc_arr = []
    for c in range(NCORES):
        for ph in (0, 1):
            gidx, counts = percore[c][ph]
            gp = np.full(Spad[ph], NPAD1 - 1, np.int64)  # pad -> all-zero p row
            gp[:len(gidx)] = gidx
            g_arr[c][ph] = _wrap_call(gp)
        cols = []
        for (ph, m_k, c16, ncol), k in zip(rounds, _round_ks(rounds)):
            A = Amats[c][ph]
            arr = np.full(ncol * 16, TRASH, np.int64)
            take = min(m_k, A.shape[0])
            arr[:take] = A[:take, k]
            arr[m_k:] = -1  # beyond num_idxs: trailing pad
            cols.append(_wrap_call(arr))
        sc_arr.append(np.concatenate(cols, axis=1) if cols
                      else np.zeros((P, 0), np.int16))
    return g_arr, sc_arr, rounds, Spad


def _round_ks(rounds):
    """Recover per-round k (edge position) from the interleaved round list."""
    kctr = {0: 0, 1: 0}
    ks = []
    for ph, m_k, c16, ncol in rounds:
        ks.append(kctr[ph])
        kctr[ph] += 1
    return ks


def _build_schedule(edge_index):
    src = np.asarray(edge_index[0], dtype=np.int64)
    dst = np.asarray(edge_index[1], dtype=np.int64)
    deg = np.bincount(dst, minlength=N).astype(np.float32)
    recip = 1.0 / np.maximum(deg, 1.0)

    # L1: dst-partitioned; window/slot from local node l = p*49 + k
    l1 = []
    for c in range(NCORES):
        m = (dst >= c * NLOC) & (dst < (c + 1) * NLOC)
        s, d = src[m], dst[m] - c * NLOC
        win = d % NW1
        slot = d // NW1
        blo = s < XSPLIT
        l1.append([
            (s[blo], win[blo], slot[blo]),
            (s[~blo] - XSPLIT, win[~blo], slot[~blo]),
        ])
    i1, d1, chunks1, R1 = _build_layer(
        l1, NW1, list(range(0, 45, L1_WCHUNK)) + [46, 48, NW1], 2)

    g_arr, sc_arr, rounds, Spad = _build_scatter(src, dst)
    return i1, d1, chunks1, R1, g_arr, sc_arr, rounds, Spad, recip


def kernel(x, edge_index, w1_l, b1, w1_r, w2_l, b2, w2_r):
    import concourse.bacc as bacc
    import concourse.mybir as mybir
    import concourse.tile as tile
    from concourse.bass_utils import run_bass_kernel_spmd
    from concourse.library_config import mlp
    from concourse.masks import make_identity

    x = np.asarray(x, np.float32)
    (i1, d1, chunks1, R1, g_arr, sc_arr, rounds, Spad,
     recip) = _build_schedule(np.asarray(edge_index))
    CRMAX = max(ch["nranks"] for ch in chunks1)

    xlo = x[:XSPLIT].astype(f16)
    xhi = x[XSPLIT:].astype(f16)
    iota_np = np.tile((np.arange(1024) % 512).astype(np.float32)[None, :],
                      (P, 1)).astype(f16)

    T16_1 = i1[0].shape[1]
    T16_S = sc_arr[0].shape[1]

    nc = bacc.Bacc("TRN2", dynamic_dma_scratch_size=49152)
    dt = mybir.dt
    t_xlo = nc.declare_dram_parameter("xlo", [XSPLIT, DIN], dt.float16, isOutput=False)
    t_xhi = nc.declare_dram_parameter("xhi", [N - XSPLIT, DIN], dt.float16, isOutput=False)
    t_i1 = nc.declare_dram_parameter("i1", [P, T16_1], dt.int16, isOutput=False)
    t_d1 = nc.declare_dram_parameter("d1", [P, R1], dt.float32, isOutput=False)
    t_gl = nc.declare_dram_parameter("gl", [P, Spad[0] // 16], dt.int16, isOutput=False)
    t_gh = nc.declare_dram_parameter("gh", [P, Spad[1] // 16], dt.int16, isOutput=False)
    t_sc = nc.declare_dram_parameter("sc", [P, T16_S], dt.int16, isOutput=False)
    t_xoT = nc.declare_dram_parameter("xoT", [DIN, NW1, P], dt.float16, isOutput=False)
    t_w1l = nc.declare_dram_parameter("w1l", [DIN, HID], dt.float16, isOutput=False)
    t_w1r = nc.declare_dram_parameter("w1r", [DIN, HID], dt.float16, isOutput=False)
    t_w2l = nc.declare_dram_parameter("w2l", [P, HID // P, OUT], dt.float16, isOutput=False)
    t_w2r = nc.declare_dram_parameter("w2r", [P, HID // P, OUT], dt.float16, isOutput=False)
    t_b1 = nc.declare_dram_parameter("b1", [P, HID // P], dt.float32, isOutput=False)
    t_b2 = nc.declare_dram_parameter("b2r", [P, OUT], dt.float32, isOutput=False)
    t_rc = nc.declare_dram_parameter("rc", [P, NW1], dt.float32, isOutput=False)
    t_rc40 = nc.declare_dram_parameter("rc40", [P, NW1, OUT], dt.float16, isOutput=False)
    t_iota = nc.declare_dram_parameter("iota", [P, 1024], dt.float16, isOutput=False)
    t_out = nc.declare_dram_parameter("out", [P, NW1, OUT], dt.float16, isOutput=True)

    t_p = nc.dram_tensor("ptab", [P, NW1, DIN], dt.float16)        # row l = p*49+k
    t_cont = nc.dram_tensor("cont", [NCORES * NPAD1, OUT], dt.float8e4)
    t_rs = nc.dram_tensor("rsout", [NPAD1, OUT], dt.float8e4)

    AluOp = mybir.AluOpType
    AF = mybir.ActivationFunctionType

    with tile.TileContext(nc) as tc:
        with tc.tile_pool(name="const", bufs=1) as cpool, \
             tc.tile_pool(name="epi", bufs=1) as epool:
            nc.gpsimd.load_library(mlp)
            ident = cpool.tile([P, P], dt.float16)
            make_identity(nc, ident[:])
            # i1 first so gather-0 starts early; big/late tables (xoT, sc,
            # gl, gh, rc40) queue after it.
            i1_t = cpool.tile([P, T16_1], dt.int16)
            nc.sync.dma_start(i1_t[:], t_i1[:])
            d1_t = cpool.tile([P, R1], dt.float32)
            nc.sync.dma_start(d1_t[:], t_d1[:])
            iota_t = cpool.tile([P, 1024], dt.float16)
            nc.sync.dma_start(iota_t[:], t_iota[:])
            w1l_t = cpool.tile([DIN, HID], dt.float16)
            nc.sync.dma_start(w1l_t[:], t_w1l[:])
            w1r_t = cpool.tile([DIN, HID], dt.float16)
            nc.sync.dma_start(w1r_t[:], t_w1r[:])
            w2l_t = cpool.tile([P, HID // P, OUT], dt.float16)
            nc.sync.dma_start(w2l_t[:], t_w2l[:])
            w2r_t = cpool.tile([P, HID // P, OUT], dt.float16)
            nc.sync.dma_start(w2r_t[:], t_w2r[:])
            b1_t = cpool.tile([P, HID // P], dt.float32)
            nc.sync.dma_start(b1_t[:], t_b1[:])
            b2_t = cpool.tile([P, OUT], dt.float32)
            nc.sync.dma_start(b2_t[:], t_b2[:])
            rc_t = cpool.tile([P, NW1], dt.float32)
            nc.sync.dma_start(rc_t[:], t_rc[:])
            # late tables: tiles allocated here, loads issued from Pool after
            # the first gather preps so they don't hog the DMA queue early
            xoT_t = cpool.tile([DIN, NW1, P], dt.float16)
            sc_t = cpool.tile([P, T16_S], dt.int16)
            gl_t = cpool.tile([P, Spad[0] // 16], dt.int16)
            gh_t = cpool.tile([P, Spad[1] // 16], dt.int16)
            rc40_t = cpool.tile([P, NW1, OUT], dt.float16)
            qbuf = cpool.tile([P, NW1, OUT], dt.float16)
            warm = cpool.tile([P, 2], dt.float32)
            nc.scalar.activation(warm[:, 0:1], b2_t[:, 0:1], AF.Exp)
            nc.scalar.activation(warm[:, 1:2], b2_t[:, 1:2], AF.Ln)
            ptA = cpool.tile([P, PTK, DIN], dt.float16)
            ptB = cpool.tile([P, PTK, DIN], dt.float16)
            nc.vector.memset(ptA[:], 0.0)
            nc.vector.memset(ptB[:], 0.0)

            # ---------------- Layer 1 ----------------
            with tc.tile_pool(name="msg", bufs=3) as mpool, \
                 tc.tile_pool(name="sm", bufs=16) as spool, \
                 tc.tile_pool(name="work", bufs=3) as wpool, \
                 tc.tile_pool(name="psumA", bufs=2, space="PSUM") as ppool, \
                 tc.tile_pool(name="psumB", bufs=2, space="PSUM") as ppoolb, \
                 tc.tile_pool(name="psumC", bufs=1, space="PSUM") as ppoolc:
                for _mb in range(3):
                    mz = mpool.tile([P, CRMAX, DIN], dt.float16, tag="msg")
                    nc.vector.memset(mz[:], 0.0)
                pt_tiles = [ptA, ptB]
                pt_pend = 0          # windows staged in current pt tile
                pt_w0 = 0
                pt_i = 0
                for ci, ch in enumerate(chunks1):
                    cr = ch["nranks"]
                    if cr == 0:
                        continue
                    msg = mpool.tile([P, CRMAX, DIN], dt.float16, tag="msg")
                    off = 0
                    for b, nidx, used, cum16 in ch["calls"]:
                        tbl = t_xlo[:] if b == 0 else t_xhi[:]
                        nc.gpsimd.dma_gather(
                            msg[:, off:off + (-(-used // P)), :], tbl,
                            i1_t[:, cum16:cum16 + used // 16],
                            used, used, DIN, single_packet=False)
                        off += nidx // P
                    if ci == 0:
                        nc.gpsimd.dma_start(xoT_t[:], t_xoT[:])
                    elif ci == 1:
                        nc.gpsimd.dma_start(sc_t[:], t_sc[:])
                    elif ci == 2:
                        nc.gpsimd.dma_start(gl_t[:], t_gl[:])
                        nc.gpsimd.dma_start(gh_t[:], t_gh[:])
                        nc.gpsimd.dma_start(rc40_t[:], t_rc40[:])
                    rank_info = {rid: (lr, q0, span) for rid, lr, q0, span in ch["ranks"]}
                    S_tiles = {}
                    for w, ops in ch["windows"]:
                        for rid, blk in ops:
                            if rid not in S_tiles:
                                lr, q0, span = rank_info[rid]
                                S = spool.tile([P, 512], dt.float16, tag="S1")
                                nc.vector.tensor_scalar(
                                    out=S[:, :span * P],
                                    in0=iota_t[:, q0 * P:(q0 + span) * P],
                                    scalar1=d1_t[:, rid:rid + 1], scalar2=None,
                                    op0=AluOp.is_equal)
                                S_tiles[rid] = S
                        pagg = ppool.tile([P, P], dt.float32, tag="pagg")
                        if not ops:
                            nc.vector.memset(pagg[:], 0.0)
                        for j, (rid, blk) in enumerate(ops):
                            lr = rank_info[rid][0]
                            nc.tensor.matmul(
                                pagg[:], lhsT=S_tiles[rid][:, blk * P:(blk + 1) * P],
                                rhs=msg[:, lr, :],
                                start=(j == 0), stop=(j == len(ops) - 1))
                        am = wpool.tile([P, P], dt.float16, tag="am")
                        nc.scalar.activation(am[:], pagg[:], AF.Copy,
                                             scale=rc_t[:, w:w + 1])
                        pamT = ppoolc.tile([P, P], dt.float16, tag="pamT")
                        nc.tensor.transpose(out=pamT[:], in_=am[:], identity=ident[:])
                        amT = wpool.tile([P, P], dt.float16, tag="amT")
                        nc.scalar.activation(amT[:], pamT[:], AF.Copy)
                        pq = ppoolc.tile([P, OUT], dt.float32, tag="pq")
                        qq = ppoolc.tile([P, OUT], dt.float32, tag="qq")
                        for bjj in range(HID // P):
                            ph_ = ppoolb.tile([P, P], dt.float32, tag="ph")
                            nc.tensor.matmul(ph_[:], lhsT=w1l_t[:, bjj * P:(bjj + 1) * P],
                                             rhs=amT[:], start=True, stop=False)
                            nc.tensor.matmul(ph_[:], lhsT=w1r_t[:, bjj * P:(bjj + 1) * P],
                                             rhs=xoT_t[:, w, :], start=False, stop=True)
                            hT = wpool.tile([P, P], dt.float16, tag="hT")
                            nc.scalar.activation(hT[:], ph_[:], AF.Relu,
                                                 bias=b1_t[:, bjj:bjj + 1])
                            nc.tensor.matmul(pq[:], lhsT=hT[:], rhs=w2l_t[:, bjj, :],
                                             start=(bjj == 0), stop=(bjj == 3))
                            nc.tensor.matmul(qq[:], lhsT=hT[:], rhs=w2r_t[:, bjj, :],
                                             start=(bjj == 0), stop=(bjj == 3))
                        nc.scalar.activation(qbuf[:, w, :], qq[:], AF.Copy)
                        pt = pt_tiles[pt_i]
                        nc.scalar.activation(pt[:, pt_pend, :OUT], pq[:], AF.Copy)
                        pt_pend += 1
                        if pt_pend == PTK or w == NW1 - 1:
                            nc.sync.dma_start(t_p[:, pt_w0:pt_w0 + pt_pend, :],
                                              pt[:, :pt_pend, :])
                            pt_w0 += pt_pend
                            pt_pend = 0
                            pt_i ^= 1

            # fold b2 into qbuf early (overlaps L2 phase / RS wait)
            nc.vector.tensor_tensor(
                out=qbuf[:], in0=qbuf[:],
                in1=b2_t[:, None, :].to_broadcast([P, NW1, OUT]), op=AluOp.add)

            # ---------------- Layer 2: SBUF parity-split scatter-add ----------
            with tc.tile_pool(name="scat", bufs=1) as zpool:
                # accumulators: [half][parity] -> [128, 99, 40] (group 98 = trash)
                acc00 = zpool.tile([P, 99, OUT], dt.float16, tag="acc00")
                acc01 = zpool.tile([P, 99, OUT], dt.float16, tag="acc01")
                acc10 = zpool.tile([P, 99, OUT], dt.float16, tag="acc10")
                acc11 = zpool.tile([P, 99, OUT], dt.float16, tag="acc11")
                accs = [[acc00, acc01], [acc10, acc11]]
                for h in (0, 1):
                    for t in (0, 1):
                        nc.vector.memset(accs[h][t][:], 0.0)
                t_p_flat = t_p[:].rearrange("p k j -> (p k) j")
                praw_l = zpool.tile([P, Spad[0] // P, DIN], dt.float16)
                nc.gpsimd.dma_gather(
                    praw_l[:], t_p_flat, gl_t[:],
                    Spad[0], Spad[0], DIN, single_packet=False)
                p_lo = zpool.tile([P, Spad[0] // P, OUT], dt.float16)
                nc.scalar.activation(p_lo[:], praw_l[:, :, :OUT], AF.Copy)
                praw_h = zpool.tile([P, Spad[1] // P, DIN], dt.float16)
                nc.gpsimd.dma_gather(
                    praw_h[:], t_p_flat, gh_t[:],
                    Spad[1], Spad[1], DIN, single_packet=False)
                p_hi = zpool.tile([P, Spad[1] // P, OUT], dt.float16)
                nc.scalar.activation(p_hi[:], praw_h[:, :, :OUT], AF.Copy)

                for ph, m_k, c16, ncol in rounds:
                    ptile = p_lo if ph == 0 else p_hi
                    nr = -(-m_k // P)
                    nc.gpsimd.dma_scatter_add(
                        accs[ph][0][:], ptile[:, :nr, :],
                        sc_t[:, c16:c16 + ncol],
                        m_k, m_k, OUT,
                        single_packet=False,
                        sbuf_tokens_per_rank=P,
                        parity_reg=0,
                        out_ap_other=accs[ph][1][:])

                # dense write-out: row 25088*h + 196*p + 98*t + g == 6272*o + dl
                tc_v = t_cont[:].rearrange("(h p q) j -> h p q j", h=2, p=P)
                for h in (0, 1):
                    for t in (0, 1):
                        acc8 = zpool.tile([P, 98, OUT], dt.float8e4,
                                          tag=f"acc8_{h}{t}")
                        nc.scalar.activation(acc8[:], accs[h][t][:, :98, :],
                                             AF.Copy)
                        nc.sync.dma_start(tc_v[h][:, 98 * t:98 * (t + 1), :],
                                          acc8[:])

                # ---------------- ReduceScatter + epilogue ----------------
                nc.gpsimd.collective_compute(
                    "ReduceScatter", AluOp.add, replica_groups=[list(range(NCORES))],
                    ins=[t_cont[:]], outs=[t_rs[:]])

                rsb = epool.tile([P, NW1, OUT], dt.float8e4, tag="rsb")
                nc.sync.dma_start(
                    rsb[:], t_rs[:].rearrange("(p k) j -> p k j", p=P))
                HALVES = [(0, 25), (25, NW1)]
                zts, ezs, ssums, lsums = [], [], [], []
                for h, (k0, k1) in enumerate(HALVES):
                    KH = k1 - k0
                    zt = epool.tile([P, KH, OUT], dt.float16, tag=f"zt{h}")
                    nc.vector.tensor_tensor(out=zt[:], in0=rsb[:, k0:k1, :],
                                            in1=rc40_t[:, k0:k1, :], op=AluOp.mult)
                    nc.vector.tensor_tensor(out=zt[:], in0=zt[:],
                                            in1=qbuf[:, k0:k1, :], op=AluOp.add)
                    zts.append(zt)
                    ez = epool.tile([P, KH, OUT], dt.float32, tag=f"ez{h}")
                    nc.scalar.activation(ez[:], zt[:], AF.Exp)
                    ezs.append(ez)
                for h, (k0, k1) in enumerate(HALVES):
                    KH = k1 - k0
                    ssum = epool.tile([P, KH, 1], dt.float32, tag=f"ssum{h}")
                    nc.vector.tensor_reduce(ssum[:], ezs[h][:],
                                            axis=mybir.AxisListType.X, op=AluOp.add)
                    ssums.append(ssum)
                for h, (k0, k1) in enumerate(HALVES):
                    KH = k1 - k0
                    lsum = epool.tile([P, KH, 1], dt.float32, tag=f"lsum{h}")
                    nc.scalar.activation(lsum[:], ssums[h][:], AF.Ln)
                    lsums.append(lsum)
                for h, (k0, k1) in enumerate(HALVES):
                    KH = k1 - k0
                    ot = epool.tile([P, KH, OUT], dt.float16, tag=f"ot{h}")
                    nc.vector.tensor_tensor(
                        out=ot[:], in0=zts[h][:],
                        in1=lsums[h][:].to_broadcast([P, KH, OUT]), op=AluOp.subtract)
                    nc.sync.dma_start(t_out[:, k0:k1, :], ot[:])

    nc.compile()

    b2b_np = np.tile(np.asarray(b2, np.float32)[None, :], (P, 1))
    in_maps = []
    for c in range(NCORES):
        xl = np.zeros((NPAD1, DIN), np.float32)
        xl[:NLOC] = x[c * NLOC:(c + 1) * NLOC]
        xoT = np.ascontiguousarray(
            xl.reshape(P, NW1, DIN).transpose(2, 1, 0)).astype(f16)
        rcf = np.ones(NPAD1, np.float32)
        rcf[:NLOC] = recip[c * NLOC:(c + 1) * NLOC]
        rcc = rcf.reshape(P, NW1).copy()
        in_maps.append({
            "xlo": xlo, "xhi": xhi,
            "i1": i1[c], "d1": d1[c],
            "gl": g_arr[c][0], "gh": g_arr[c][1], "sc": sc_arr[c],
            "xoT": xoT, "rc": rcc, "iota": iota_np,
            "rc40": np.ascontiguousarray(np.broadcast_to(rcc[:, :, None], (P, NW1, OUT))).astype(f16),
            "w1l": np.asarray(w1_l).astype(f16),
            "w1r": np.asarray(w1_r).astype(f16),
            "w2l": np.ascontiguousarray(
                np.asarray(w2_l).astype(f16).reshape(HID // P, P, OUT)
                .transpose(1, 0, 2)),
            "w2r": np.ascontiguousarray(
                np.asarray(w2_r).astype(f16).reshape(HID // P, P, OUT)
                .transpose(1, 0, 2)),
            "b1": np.asarray(b1, np.float32).reshape(HID // P, P).T.copy(),
            "b2r": b2b_np,
        })
    res = run_bass_kernel_spmd(nc, in_maps, list(range(NCORES)))
    out = np.concatenate(
        [res.results[c]["out"].reshape(NPAD1, OUT)[:NLOC] for c in range(NCORES)],
        axis=0)
    kernel.last_results = res
    kernel.last_nc = nc
    return out.astype(np.float32)


# revision 44
# speedup vs baseline: 1.1406x; 1.0068x over previous
"""GraphSAGE (2-layer, mean-agg) Trainium2 Bass kernel, 8-core SPMD.

Design v3 (scatter-add L2):
- L1 dst-partitioned: each core owns 6250 dst nodes; fp16 x tables (xlo/xhi)
  replicated in DRAM; per-edge messages fetched with gpsimd dma_gather;
  segment-sum on the PE via per-rank selection matmuls.  S matrices are
  built on DVE with one tensor_scalar is_equal per rank against a fp16
  iota table; per-window dense compute yields p = h@w2_l and q = h@w2_r
  (40-dim each).  p rows are written to DRAM t_p (256B rows, node-id
  indexed).
- L2 src-partitioned via gpsimd dma_scatter_add: per-edge messages are the
  40-fp16 (80B) p rows, far below dma_gather's 256B-row minimum, so each
  edge costs ~7ns of descriptor time instead of ~22.8ns.  The core's own
  p rows are loaded into SBUF twice (dma_gather from t_p), sorted by the
  node's lo-half / hi-half out-degree; scatter "rounds" then push the k-th
  edge of every still-active node in one call, using count-sorted slot
  order so only trailing slots go inactive (the scatter API only skips
  trailing indices).  Nodes with degree > RCAP get ceil(deg/RCAP)
  replicated slots so RCAP rounds always suffice.
- int16 scatter indices cap the target space at 32768 rows, so the padded
  dst space (50176 rows) is split into lo/hi halves (25088 rows each, plus
  one trash row per half that absorbs cross-core padding slots).  lo/hi
  rounds are interleaved so the two write chains pipeline.
- The only collective is one ReduceScatter(add) of the [2,25088,40] fp16
  partial view (trash rows skipped via a strided AP) -> [6272,40] per core.
- Epilogue (recip-scale, +q(+b2 prefolded), log_softmax) runs once,
  batched fp16 [128,49,40], with Exp/Ln activation tables prewarmed.
"""
import numpy as np
import ml_dtypes

N = 50000
E = 800000
DIN, HID, OUT = 128, 512, 40
NCORES = 8
NLOC = N // NCORES            # 6250
P = 128
NW1 = 49                      # L1 windows (own nodes), node l = p*49 + k
NPAD1 = P * NW1               # 6272
XSPLIT = 32768                # x table split for int16 gather indices
L1_WCHUNK = 4                 # L1 windows per gather chunk
PTK = 16                      # L1 p-table write batch (windows)
PADVAL = 600.0                # drel pad: never matches iota (0..511)
RCAP = 10                     # scatter rounds cap per half
HALF = NPAD1 * (NCORES // 2)  # 25088 rows per half
TRASH = HALF                  # trash row index within each half view
NPZ_ROWS = 2 * (HALF + 1)     # 50178 rows in the scatter target

f16 = np.float16


def _wrap_call(flat_idx):
    """int16 wrapped layout for one gather call: slot i -> [i%16, i//16]."""
    n = len(flat_idx)
    w = flat_idx.astype(np.int16).reshape(n // 16, 16).T.copy()
    return np.tile(w, (8, 1))  # [128, n/16]


def _build_layer(per_core, nwin, wchunk, nbuckets):
    """Dense-rank schedule for L1 (see v2 docstring)."""
    srt = []
    for c in range(NCORES):
        row = []
        for b in range(nbuckets):
            idx, win, slot = per_core[c][b]
            o = np.argsort(win, kind="stable")
            row.append((idx[o], win[o], slot[o]))
        srt.append(row)

    chunks = []
    idx_cols = [[] for _ in range(NCORES)]
    drel_cols = [[] for _ in range(NCORES)]
    rank_id = 0
    cum16 = 0
    if isinstance(wchunk, list):
        bounds = wchunk
    else:
        bounds = list(range(0, nwin, wchunk)) + [nwin]
    for ci, (w0, w1) in enumerate(zip(bounds[:-1], bounds[1:])):
        calls = []
        ranks_meta = []
        win_ops = {w: [] for w in range(w0, w1)}
        msg_off = 0
        for b in range(nbuckets):
            sel = []
            cnts = []
            for c in range(NCORES):
                idx, win, slot = srt[c][b]
                lo = np.searchsorted(win, w0, "left")
                hi = np.searchsorted(win, w1, "left")
                sel.append((idx[lo:hi], win[lo:hi], slot[lo:hi]))
                cnts.append(hi - lo)
            nr = (max(cnts) + P - 1) // P
            if nr == 0:
                continue
            nidx = nr * P
            # trimmed transfer count (x16); msg buffers are zeroed once up
            # front so un-transferred tail slots stay finite.
            used = -(-max(cnts) // 16) * 16
            cover = np.full((nr, 2), -1, np.int64)
            for c in range(NCORES):
                idx, win, slot = sel[c]
                flat = np.zeros(used, np.int64)
                drl = np.full(nidx, PADVAL, np.float64)
                ne = len(idx)
                flat[:ne] = idx
                drl[:ne] = slot + 128 * (win & 3)
                idx_cols[c].append(_wrap_call(flat))
                drel_cols[c].append(drl.reshape(nr, P).T.astype(np.float32))
                for r in range(nr):
                    a, z = r * P, min((r + 1) * P, ne)
                    if a >= ne:
                        break
                    wmin, wmax = win[a], win[z - 1]
                    if cover[r, 0] < 0:
                        cover[r] = (wmin, wmax)
                    else:
                        cover[r, 0] = min(cover[r, 0], wmin)
                        cover[r, 1] = max(cover[r, 1], wmax)
            calls.append((b, nidx, used, cum16))
            cum16 += used // 16
            for r in range(nr):
                wmin, wmax = cover[r]
                if wmin < 0:
                    continue
                span = int(wmax - wmin + 1)
                assert span <= 4, f"rank spans {span} windows"
                rid = rank_id + r
                ranks_meta.append((rid, msg_off + r, int(wmin) & 3, span))
                for w in range(int(wmin), int(wmax) + 1):
                    win_ops[w].append((rid, w - int(wmin)))
            rank_id += nr
            msg_off += nr
        chunks.append({
            "calls": calls,
            "nranks": msg_off,
            "ranks": ranks_meta,
            "windows": [(w, win_ops[w]) for w in range(w0, w1)],
        })
    idx_arr = [np.concatenate(idx_cols[c], axis=1) if idx_cols[c]
               else np.zeros((P, 0), np.int16) for c in range(NCORES)]
    drel_arr = [np.concatenate(drel_cols[c], axis=1) if drel_cols[c]
                else np.zeros((P, 0), np.float32) for c in range(NCORES)]
    return idx_arr, drel_arr, chunks, rank_id


def _enc_idx(d):
    """Scatter token id for global dst d: partition/slot encoding whose
    parity-split SBUF accumulators reassemble densely into row 6272*o+dl."""
    o = d // NLOC
    dl = d % NLOC
    prel = dl // 196
    q = dl % 196
    t = q // 98
    g = q % 98
    s = 2 * g + t
    return (o // 4), 128 * s + 32 * (o % 4) + prel  # (half, idx)


def _build_scatter(src, dst):
    """L2 scatter-add schedule (src-partitioned, lo/hi dst halves).

    Returns per-core gather/scatter idx tables plus the shared round
    structure: rounds = [(phase, m_k, col16, ncol16)], Spad[2].
    """
    percore = []  # [c][ph] = (gidx_sorted, counts_sorted)
    Amats = []    # [c][ph] = slot-major [S, RCAP] edge-target idxs (TRASH pad)
    for c in range(NCORES):
        m = (src >= c * NLOC) & (src < (c + 1) * NLOC)
        s_all = src[m] - c * NLOC
        d = dst[m]
        half_all, idx_all = _enc_idx(d)
        ph_data = []
        ph_A = []
        for ph in (0, 1):
            sel = half_all == ph
            ss = s_all[sel]
            rr = idx_all[sel]
            o = np.argsort(ss, kind="stable")
            ss, rr = ss[o], rr[o]
            deg = np.bincount(ss, minlength=NLOC)
            nz = np.nonzero(deg)[0]
            gidx, lists = [], []
            pos = 0
            for i in nz:
                dg = int(deg[i])
                ci = -(-dg // RCAP)
                lst = rr[pos:pos + dg]
                for j in range(ci):
                    gidx.append(i)
                    lists.append(lst[j::ci])
                pos += dg
            counts = np.fromiter((len(x) for x in lists), np.int64, len(lists))
            order = np.argsort(-counts, kind="stable")
            gidx = np.asarray(gidx, np.int64)[order]
            counts = counts[order]
            A = np.full((len(lists), RCAP), TRASH, np.int64)
            for q, oi in enumerate(order):
                A[q, :counts[q]] = lists[oi]
            ph_data.append((gidx, counts))
            ph_A.append(A)
        percore.append(ph_data)
        Amats.append(ph_A)

    Spad = []
    for ph in (0, 1):
        smax = max(len(percore[c][ph][0]) for c in range(NCORES))
        Spad.append(-(-smax // P) * P)

    rounds = []
    col16 = 0
    for k in range(RCAP):
        for ph in (0, 1):
            m_k = max(int((percore[c][ph][1] > k).sum()) for c in range(NCORES))
            if m_k == 0:
                continue
            ncol = -(-m_k // 16)
            rounds.append((ph, m_k, col16, ncol))
            col16 += ncol

    g_arr = [[None, None] for _ in range(NCORES)]
    sc_arr = []
    for c in range(NCORES):
        for ph in (0, 1):
            gidx, counts = percore[c][ph]
            gp = np.full(Spad[ph], NPAD1 - 1, np.int64)  # pad -> all-zero p row
            gp[:len(gidx)] = gidx
            g_arr[c][ph] = _wrap_call(gp)
        cols = []
        for (ph, m_k, c16, ncol), k in zip(rounds, _round_ks(rounds)):
            A = Amats[c][ph]
            arr = np.full(ncol * 16, TRASH, np.int64)
            take = min(m_k, A.shape[0])
            arr[:take] = A[:take, k]
            arr[m_k:] = -1  # beyond num_idxs: trailing pad
            cols.append(_wrap_call(arr))
        sc_arr.append(np.concatenate(cols, axis=1) if cols
                      else np.zeros((P, 0), np.int16))
    return g_arr, sc_arr, rounds, Spad


def _round_ks(rounds):
    """Recover per-round k (edge position) from the interleaved round list."""
    kctr = {0: 0, 1: 0}
    ks = []
    for ph, m_k, c16, ncol in rounds:
        ks.append(kctr[ph])
        kctr[ph] += 1
    return ks


def _build_schedule(edge_index):
    src = np.asarray(edge_index[0], dtype=np.int64)
    dst = np.asarray(edge_index[1], dtype=np.int64)
    deg = np.bincount(dst, minlength=N).astype(np.float32)
    recip = 1.0 / np.maximum(deg, 1.0)

    # L1: dst-partitioned; window/slot from local node l = p*49 + k
    l1 = []
    for c in range(NCORES):
        m = (dst >= c * NLOC) & (dst < (c + 1) * NLOC)
        s, d = src[m], dst[m] - c * NLOC
        win = d % NW1
        slot = d // NW1
        blo = s < XSPLIT
        l1.append([
            (s[blo], win[blo], slot[blo]),
            (s[~blo] - XSPLIT, win[~blo], slot[~blo]),
        ])
    i1, d1, chunks1, R1 = _build_layer(
        l1, NW1, list(range(0, 45, L1_WCHUNK)) + [46, 48, NW1], 2)

    g_arr, sc_arr, rounds, Spad = _build_scatter(src, dst)
    return i1, d1, chunks1, R1, g_arr, sc_arr, rounds, Spad, recip


def kernel(x, edge_index, w1_l, b1, w1_r, w2_l, b2, w2_r):
    import concourse.bacc as bacc
    import concourse.mybir as mybir
    import concourse.tile as tile
    from concourse.bass_utils import run_bass_kernel_spmd
    from concourse.library_config import mlp
    from concourse.masks import make_identity

    x = np.asarray(x, np.float32)
    (i1, d1, chunks1, R1, g_arr, sc_arr, rounds, Spad,
     recip) = _build_schedule(np.asarray(edge_index))
    CRMAX = max(ch["nranks"] for ch in chunks1)

    xlo = x[:XSPLIT].astype(f16)
    xhi = x[XSPLIT:].astype(f16)
    iota_np = np.tile((np.arange(1024) % 512).astype(np.float32)[None, :],
                      (P, 1)).astype(f16)

    T16_1 = i1[0].shape[1]
    T16_S = sc_arr[0].shape[1]

    nc = bacc.Bacc("TRN2", dynamic_dma_scratch_size=49152)
    dt = mybir.dt
    t_xlo = nc.declare_dram_parameter("xlo", [XSPLIT, DIN], dt.float16, isOutput=False)
    t_xhi = nc.declare_dram_parameter("xhi", [N - XSPLIT, DIN], dt.float16, isOutput=False)
    t_i1 = nc.declare_dram_parameter("i1", [P, T16_1], dt.int16, isOutput=False)
    t_d1 = nc.declare_dram_parameter("d1", [P, R1], dt.float32, isOutput=False)
    t_gl = nc.declare_dram_parameter("gl", [P, Spad[0] // 16], dt.int16, isOutput=False)
    t_gh = nc.declare_dram_parameter("gh", [P, Spad[1] // 16], dt.int16, isOutput=False)
    t_sc = nc.declare_dram_parameter("sc", [P, T16_S], dt.int16, isOutput=False)
    t_xoT = nc.declare_dram_parameter("xoT", [DIN, NW1, P], dt.float16, isOutput=False)
    t_w1l = nc.declare_dram_parameter("w1l", [DIN, HID], dt.float16, isOutput=False)
    t_w1r = nc.declare_dram_parameter("w1r", [DIN, HID], dt.float16, isOutput=False)
    t_w2l = nc.declare_dram_parameter("w2l", [P, HID // P, OUT], dt.float16, isOutput=False)
    t_w2r = nc.declare_dram_parameter("w2r", [P, HID // P, OUT], dt.float16, isOutput=False)
    t_b1 = nc.declare_dram_parameter("b1", [P, HID // P], dt.float32, isOutput=False)
    t_b2 = nc.declare_dram_parameter("b2r", [P, OUT], dt.float32, isOutput=False)
    t_rc = nc.declare_dram_parameter("rc", [P, NW1], dt.float32, isOutput=False)
    t_rc40 = nc.declare_dram_parameter("rc40", [P, NW1, OUT], dt.float16, isOutput=False)
    t_iota = nc.declare_dram_parameter("iota", [P, 1024], dt.float16, isOutput=False)
    t_out = nc.declare_dram_parameter("out", [P, NW1, OUT], dt.float16, isOutput=True)

    t_p = nc.dram_tensor("ptab", [P, NW1, DIN], dt.float16)        # row l = p*49+k
    t_cont = nc.dram_tensor("cont", [NCORES * NPAD1, OUT], dt.float8e4)
    t_rs = nc.dram_tensor("rsout", [NPAD1, OUT], dt.float8e4)

    AluOp = mybir.AluOpType
    AF = mybir.ActivationFunctionType

    with tile.TileContext(nc) as tc:
        with tc.tile_pool(name="const", bufs=1) as cpool, \
             tc.tile_pool(name="epi", bufs=1) as epool:
            nc.gpsimd.load_library(mlp)
            ident = cpool.tile([P, P], dt.float16)
            make_identity(nc, ident[:])
            # i1 first so gather-0 starts early; big/late tables (xoT, sc,
            # gl, gh, rc40) queue after it.
            i1_t = cpool.tile([P, T16_1], dt.int16)
            nc.sync.dma_start(i1_t[:], t_i1[:])
            d1_t = cpool.tile([P, R1], dt.float32)
            nc.sync.dma_start(d1_t[:], t_d1[:])
            iota_t = cpool.tile([P, 1024], dt.float16)
            nc.sync.dma_start(iota_t[:], t_iota[:])
            w1l_t = cpool.tile([DIN, HID], dt.float16)
            nc.sync.dma_start(w1l_t[:], t_w1l[:])
            w1r_t = cpool.tile([DIN, HID], dt.float16)
            nc.sync.dma_start(w1r_t[:], t_w1r[:])
            w2l_t = cpool.tile([P, HID // P, OUT], dt.float16)
            nc.sync.dma_start(w2l_t[:], t_w2l[:])
            w2r_t = cpool.tile([P, HID // P, OUT], dt.float16)
            nc.sync.dma_start(w2r_t[:], t_w2r[:])
            b1_t = cpool.tile([P, HID // P], dt.float32)
            nc.sync.dma_start(b1_t[:], t_b1[:])
            b2_t = cpool.tile([P, OUT], dt.float32)
            nc.sync.dma_start(b2_t[:], t_b2[:])
            rc_t = cpool.tile([P, NW1], dt.float32)
            nc.sync.dma_start(rc_t[:], t_rc[:])
            # late tables: tiles allocated here, loads issued from Pool after
            # the first gather preps so they don't hog the DMA queue early
            xoT_t = cpool.tile([DIN, NW1, P], dt.float16)
            sc_t = cpool.tile([P, T16_S], dt.int16)
            gl_t = cpool.tile([P, Spad[0] // 16], dt.int16)
            gh_t = cpool.tile([P, Spad[1] // 16], dt.int16)
            rc40_t = cpool.tile([P, NW1, OUT], dt.float16)
            qbuf = cpool.tile([P, NW1, OUT], dt.float16)
            warm = cpool.tile([P, 2], dt.float32)
            nc.scalar.activation(warm[:, 0:1], b2_t[:, 0:1], AF.Exp)
            nc.scalar.activation(warm[:, 1:2], b2_t[:, 1:2], AF.Ln)
            ptA = cpool.tile([P, PTK, DIN], dt.float16)
            ptB = cpool.tile([P, PTK, DIN], dt.float16)
            nc.vector.memset(ptA[:], 0.0)
            nc.vector.memset(ptB[:], 0.0)

            # ---------------- Layer 1 ----------------
            with tc.tile_pool(name="msg", bufs=3) as mpool, \
                 tc.tile_pool(name="sm", bufs=16) as spool, \
                 tc.tile_pool(name="work", bufs=3) as wpool, \
                 tc.tile_pool(name="psumA", bufs=2, space="PSUM") as ppool, \
                 tc.tile_pool(name="psumB", bufs=2, space="PSUM") as ppoolb, \
                 tc.tile_pool(name="psumC", bufs=1, space="PSUM") as ppoolc:
                for _mb in range(3):
                    mz = mpool.tile([P, CRMAX, DIN], dt.float16, tag="msg")
                    nc.vector.memset(mz[:], 0.0)
                pt_tiles = [ptA, ptB]
                pt_pend = 0          # windows staged in current pt tile
                pt_w0 = 0
                pt_i = 0
                for ci, ch in enumerate(chunks1):
                    cr = ch["nranks"]
                    if cr == 0:
                        continue
                    msg = mpool.tile([P, CRMAX, DIN], dt.float16, tag="msg")
                    off = 0
                    for b, nidx, used, cum16 in ch["calls"]:
                        tbl = t_xlo[:] if b == 0 else t_xhi[:]
                        nc.gpsimd.dma_gather(
                            msg[:, off:off + (-(-used // P)), :], tbl,
                            i1_t[:, cum16:cum16 + used // 16],
                            used, used, DIN, single_packet=False)
                        off += nidx // P
                    if ci == 0:
                        nc.gpsimd.dma_start(xoT_t[:], t_xoT[:])
                    elif ci == 1:
                        nc.gpsimd.dma_start(sc_t[:], t_sc[:])
                    elif ci == 2:
                        nc.gpsimd.dma_start(gl_t[:], t_gl[:])
                        nc.gpsimd.dma_start(gh_t[:], t_gh[:])
                        nc.gpsimd.dma_start(rc40_t[:], t_rc40[:])
                    rank_info = {rid: (lr, q0, span) for rid, lr, q0, span in ch["ranks"]}
                    S_tiles = {}
                    for w, ops in ch["windows"]:
                        for rid, blk in ops:
                            if rid not in S_tiles:
                                lr, q0, span = rank_info[rid]
                                S = spool.tile([P, 512], dt.float16, tag="S1")
                                nc.vector.tensor_scalar(
                                    out=S[:, :span * P],
                                    in0=iota_t[:, q0 * P:(q0 + span) * P],
                                    scalar1=d1_t[:, rid:rid + 1], scalar2=None,
                                    op0=AluOp.is_equal)
                                S_tiles[rid] = S
                        pagg = ppool.tile([P, P], dt.float32, tag="pagg")
                        if not ops:
                            nc.vector.memset(pagg[:], 0.0)
                        for j, (rid, blk) in enumerate(ops):
                            lr = rank_info[rid][0]
                            nc.tensor.matmul(
                                pagg[:], lhsT=S_tiles[rid][:, blk * P:(blk + 1) * P],
                                rhs=msg[:, lr, :],
                                start=(j == 0), stop=(j == len(ops) - 1))
                        am = wpool.tile([P, P], dt.float16, tag="am")
                        nc.scalar.activation(am[:], pagg[:], AF.Copy,
                                             scale=rc_t[:, w:w + 1])
                        pamT = ppoolc.tile([P, P], dt.float16, tag="pamT")
                        nc.tensor.transpose(out=pamT[:], in_=am[:], identity=ident[:])
                        amT = wpool.tile([P, P], dt.float16, tag="amT")
                        nc.scalar.activation(amT[:], pamT[:], AF.Copy)
                        pq = ppoolc.tile([P, OUT], dt.float32, tag="pq")
                        qq = ppoolc.tile([P, OUT], dt.float32, tag="qq")
                        for bjj in range(HID // P):
                            ph_ = ppoolb.tile([P, P], dt.float32, tag="ph")
                            nc.tensor.matmul(ph_[:], lhsT=w1l_t[:, bjj * P:(bjj + 1) * P],
                                             rhs=amT[:], start=True, stop=False)
                            nc.tensor.matmul(ph_[:], lhsT=w1r_t[:, bjj * P:(bjj + 1) * P],
                                             rhs=xoT_t[:, w, :], start=False, stop=True)
                            hT = wpool.tile([P, P], dt.float16, tag="hT")
                            nc.scalar.activation(hT[:], ph_[:], AF.Relu,
                                                 bias=b1_t[:, bjj:bjj + 1])
                            nc.tensor.matmul(pq[:], lhsT=hT[:], rhs=w2l_t[:, bjj, :],
                                             start=(bjj == 0), stop=(bjj == 3))
                            nc.tensor.matmul(qq[:], lhsT=hT[:], rhs=w2r_t[:, bjj, :],
                                             start=(bjj == 0), stop=(bjj == 3))
                        nc.scalar.activation(qbuf[:, w, :], qq[:], AF.Copy)
                        pt = pt_tiles[pt_i]
                        nc.scalar.activation(pt[:, pt_pend, :OUT], pq[:], AF.Copy)
                        pt_pend += 1
                        if (pt_pend == PTK or w == NW1 - 1
                                or (w >= 31 and pt_pend >= 8)):
                            nc.sync.dma_start(t_p[:, pt_w0:pt_w0 + pt_pend, :],
                                              pt[:, :pt_pend, :])
                            pt_w0 += pt_pend
                            pt_pend = 0
                            pt_i ^= 1

            # fold b2 into qbuf early (overlaps L2 phase / RS wait)
            nc.vector.tensor_tensor(
                out=qbuf[:], in0=qbuf[:],
                in1=b2_t[:, None, :].to_broadcast([P, NW1, OUT]), op=AluOp.add)

            # ---------------- Layer 2: SBUF parity-split scatter-add ----------
            with tc.tile_pool(name="scat", bufs=1) as zpool:
                # accumulators: [half][parity] -> [128, 99, 40] (group 98 = trash)
                acc00 = zpool.tile([P, 99, OUT], dt.float16, tag="acc00")
                acc01 = zpool.tile([P, 99, OUT], dt.float16, tag="acc01")
                acc10 = zpool.tile([P, 99, OUT], dt.float16, tag="acc10")
                acc11 = zpool.tile([P, 99, OUT], dt.float16, tag="acc11")
                accs = [[acc00, acc01], [acc10, acc11]]
                for h in (0, 1):
                    for t in (0, 1):
                        nc.vector.memset(accs[h][t][:], 0.0)
                t_p_flat = t_p[:].rearrange("p k j -> (p k) j")
                praw_l = zpool.tile([P, Spad[0] // P, DIN], dt.float16)
                nc.gpsimd.dma_gather(
                    praw_l[:], t_p_flat, gl_t[:],
                    Spad[0], Spad[0], DIN, single_packet=False)
                p_lo = zpool.tile([P, Spad[0] // P, OUT], dt.float16)
                nc.scalar.activation(p_lo[:], praw_l[:, :, :OUT], AF.Copy)
                praw_h = zpool.tile([P, Spad[1] // P, DIN], dt.float16)
                nc.gpsimd.dma_gather(
                    praw_h[:], t_p_flat, gh_t[:],
                    Spad[1], Spad[1], DIN, single_packet=False)
                p_hi = zpool.tile([P, Spad[1] // P, OUT], dt.float16)
                nc.scalar.activation(p_hi[:], praw_h[:, :, :OUT], AF.Copy)

                for ph, m_k, c16, ncol in rounds:
                    ptile = p_lo if ph == 0 else p_hi
                    nr = -(-m_k // P)
                    nc.gpsimd.dma_scatter_add(
                        accs[ph][0][:], ptile[:, :nr, :],
                        sc_t[:, c16:c16 + ncol],
                        m_k, m_k, OUT,
                        single_packet=False,
                        sbuf_tokens_per_rank=P,
                        parity_reg=0,
                        out_ap_other=accs[ph][1][:])

                # dense write-out: row 25088*h + 196*p + 98*t + g == 6272*o + dl
                tc_v = t_cont[:].rearrange("(h p q) j -> h p q j", h=2, p=P)
                for h in (0, 1):
                    for t in (0, 1):
                        acc8 = zpool.tile([P, 98, OUT], dt.float8e4,
                                          tag=f"acc8_{h}{t}")
                        nc.scalar.activation(acc8[:], accs[h][t][:, :98, :],
                                             AF.Copy)
                        nc.sync.dma_start(tc_v[h][:, 98 * t:98 * (t + 1), :],
                                          acc8[:])

                # ---------------- ReduceScatter + epilogue ----------------
                nc.gpsimd.collective_compute(
                    "ReduceScatter", AluOp.add, replica_groups=[list(range(NCORES))],
                    ins=[t_cont[:]], outs=[t_rs[:]])

                rsb = epool.tile([P, NW1, OUT], dt.float8e4, tag="rsb")
                nc.sync.dma_start(
                    rsb[:], t_rs[:].rearrange("(p k) j -> p k j", p=P))
                HALVES = [(0, 25), (25, NW1)]
                zts, ezs, ssums, lsums = [], [], [], []
                for h, (k0, k1) in enumerate(HALVES):
                    KH = k1 - k0
                    zt = epool.tile([P, KH, OUT], dt.float16, tag=f"zt{h}")
                    nc.vector.tensor_tensor(out=zt[:], in0=rsb[:, k0:k1, :],
                                            in1=rc40_t[:, k0:k1, :], op=AluOp.mult)
                    nc.vector.tensor_tensor(out=zt[:], in0=zt[:],
                                            in1=qbuf[:, k0:k1, :], op=AluOp.add)
                    zts.append(zt)
                    ez = epool.tile([P, KH, OUT], dt.float32, tag=f"ez{h}")
                    nc.scalar.activation(ez[:], zt[:], AF.Exp)
                    ezs.append(ez)
                for h, (k0, k1) in enumerate(HALVES):
                    KH = k1 - k0
                    ssum = epool.tile([P, KH, 1], dt.float32, tag=f"ssum{h}")
                    nc.vector.tensor_reduce(ssum[:], ezs[h][:],
                                            axis=mybir.AxisListType.X, op=AluOp.add)
                    ssums.append(ssum)
                for h, (k0, k1) in enumerate(HALVES):
                    KH = k1 - k0
                    lsum = epool.tile([P, KH, 1], dt.float32, tag=f"lsum{h}")
                    nc.scalar.activation(lsum[:], ssums[h][:], AF.Ln)
                    lsums.append(lsum)
                for h, (k0, k1) in enumerate(HALVES):
                    KH = k1 - k0
                    ot = epool.tile([P, KH, OUT], dt.float16, tag=f"ot{h}")
                    nc.vector.tensor_tensor(
                        out=ot[:], in0=zts[h][:],
                        in1=lsums[h][:].to_broadcast([P, KH, OUT]), op=AluOp.subtract)
                    nc.sync.dma_start(t_out[:, k0:k1, :], ot[:])

    nc.compile()

    b2b_np = np.tile(np.asarray(b2, np.float32)[None, :], (P, 1))
    in_maps = []
    for c in range(NCORES):
        xl = np.zeros((NPAD1, DIN), np.float32)
        xl[:NLOC] = x[c * NLOC:(c + 1) * NLOC]
        xoT = np.ascontiguousarray(
            xl.reshape(P, NW1, DIN).transpose(2, 1, 0)).astype(f16)
        rcf = np.ones(NPAD1, np.float32)
        rcf[:NLOC] = recip[c * NLOC:(c + 1) * NLOC]
        rcc = rcf.reshape(P, NW1).copy()
        in_maps.append({
            "xlo": xlo, "xhi": xhi,
            "i1": i1[c], "d1": d1[c],
            "gl": g_arr[c][0], "gh": g_arr[c][1], "sc": sc_arr[c],
            "xoT": xoT, "rc": rcc, "iota": iota_np,
            "rc40": np.ascontiguousarray(np.broadcast_to(rcc[:, :, None], (P, NW1, OUT))).astype(f16),
            "w1l": np.asarray(w1_l).astype(f16),
            "w1r": np.asarray(w1_r).astype(f16),
            "w2l": np.ascontiguousarray(
                np.asarray(w2_l).astype(f16).reshape(HID // P, P, OUT)
                .transpose(1, 0, 2)),
            "w2r": np.ascontiguousarray(
                np.asarray(w2_r).astype(f16).reshape(HID // P, P, OUT)
                .transpose(1, 0, 2)),
            "b1": np.asarray(b1, np.float32).reshape(HID // P, P).T.copy(),
            "b2r": b2b_np,
        })
    res = run_bass_kernel_spmd(nc, in_maps, list(range(NCORES)))
    out = np.concatenate(
        [res.results[c]["out"].reshape(NPAD1, OUT)[:NLOC] for c in range(NCORES)],
        axis=0)
    kernel.last_results = res
    kernel.last_nc = nc
    return out.astype(np.float32)


# revision 45
# speedup vs baseline: 1.1414x; 1.0008x over previous
"""GraphSAGE (2-layer, mean-agg) Trainium2 Bass kernel, 8-core SPMD.

Design v3 (scatter-add L2):
- L1 dst-partitioned: each core owns 6250 dst nodes; fp16 x tables (xlo/xhi)
  replicated in DRAM; per-edge messages fetched with gpsimd dma_gather;
  segment-sum on the PE via per-rank selection matmuls.  S matrices are
  built on DVE with one tensor_scalar is_equal per rank against a fp16
  iota table; per-window dense compute yields p = h@w2_l and q = h@w2_r
  (40-dim each).  p rows are written to DRAM t_p (256B rows, node-id
  indexed).
- L2 src-partitioned via gpsimd dma_scatter_add: per-edge messages are the
  40-fp16 (80B) p rows, far below dma_gather's 256B-row minimum, so each
  edge costs ~7ns of descriptor time instead of ~22.8ns.  The core's own
  p rows are loaded into SBUF twice (dma_gather from t_p), sorted by the
  node's lo-half / hi-half out-degree; scatter "rounds" then push the k-th
  edge of every still-active node in one call, using count-sorted slot
  order so only trailing slots go inactive (the scatter API only skips
  trailing indices).  Nodes with degree > RCAP get ceil(deg/RCAP)
  replicated slots so RCAP rounds always suffice.
- int16 scatter indices cap the target space at 32768 rows, so the padded
  dst space (50176 rows) is split into lo/hi halves (25088 rows each, plus
  one trash row per half that absorbs cross-core padding slots).  lo/hi
  rounds are interleaved so the two write chains pipeline.
- The only collective is one ReduceScatter(add) of the [2,25088,40] fp16
  partial view (trash rows skipped via a strided AP) -> [6272,40] per core.
- Epilogue (recip-scale, +q(+b2 prefolded), log_softmax) runs once,
  batched fp16 [128,49,40], with Exp/Ln activation tables prewarmed.
"""
import numpy as np
import ml_dtypes

N = 50000
E = 800000
DIN, HID, OUT = 128, 512, 40
NCORES = 8
NLOC = N // NCORES            # 6250
P = 128
NW1 = 49                      # L1 windows (own nodes), node l = p*49 + k
NPAD1 = P * NW1               # 6272
XSPLIT = 32768                # x table split for int16 gather indices
L1_WCHUNK = 4                 # L1 windows per gather chunk
PTK = 16                      # L1 p-table write batch (windows)
PADVAL = 600.0                # drel pad: never matches iota (0..511)
RCAP = 10                     # scatter rounds cap per half
HALF = NPAD1 * (NCORES // 2)  # 25088 rows per half
TRASH = HALF                  # trash row index within each half view
NPZ_ROWS = 2 * (HALF + 1)     # 50178 rows in the scatter target

f16 = np.float16


def _wrap_call(flat_idx):
    """int16 wrapped layout for one gather call: slot i -> [i%16, i//16]."""
    n = len(flat_idx)
    w = flat_idx.astype(np.int16).reshape(n // 16, 16).T.copy()
    return np.tile(w, (8, 1))  # [128, n/16]


def _build_layer(per_core, nwin, wchunk, nbuckets):
    """Dense-rank schedule for L1 (see v2 docstring)."""
    srt = []
    for c in range(NCORES):
        row = []
        for b in range(nbuckets):
            idx, win, slot = per_core[c][b]
            o = np.argsort(win, kind="stable")
            row.append((idx[o], win[o], slot[o]))
        srt.append(row)

    chunks = []
    idx_cols = [[] for _ in range(NCORES)]
    drel_cols = [[] for _ in range(NCORES)]
    rank_id = 0
    cum16 = 0
    if isinstance(wchunk, list):
        bounds = wchunk
    else:
        bounds = list(range(0, nwin, wchunk)) + [nwin]
    for ci, (w0, w1) in enumerate(zip(bounds[:-1], bounds[1:])):
        calls = []
        ranks_meta = []
        win_ops = {w: [] for w in range(w0, w1)}
        msg_off = 0
        for b in range(nbuckets):
            sel = []
            cnts = []
            for c in range(NCORES):
                idx, win, slot = srt[c][b]
                lo = np.searchsorted(win, w0, "left")
                hi = np.searchsorted(win, w1, "left")
                sel.append((idx[lo:hi], win[lo:hi], slot[lo:hi]))
                cnts.append(hi - lo)
            nr = (max(cnts) + P - 1) // P
            if nr == 0:
                continue
            nidx = nr * P
            # trimmed transfer count (x16); msg buffers are zeroed once up
            # front so un-transferred tail slots stay finite.
            used = -(-max(cnts) // 16) * 16
            cover = np.full((nr, 2), -1, np.int64)
            for c in range(NCORES):
                idx, win, slot = sel[c]
                flat = np.zeros(used, np.int64)
                drl = np.full(nidx, PADVAL, np.float64)
                ne = len(idx)
                flat[:ne] = idx
                drl[:ne] = slot + 128 * (win & 3)
                idx_cols[c].append(_wrap_call(flat))
                drel_cols[c].append(drl.reshape(nr, P).T.astype(np.float32))
                for r in range(nr):
                    a, z = r * P, min((r + 1) * P, ne)
                    if a >= ne:
                        break
                    wmin, wmax = win[a], win[z - 1]
                    if cover[r, 0] < 0:
                        cover[r] = (wmin, wmax)
                    else:
                        cover[r, 0] = min(cover[r, 0], wmin)
                        cover[r, 1] = max(cover[r, 1], wmax)
            calls.append((b, nidx, used, cum16))
            cum16 += used // 16
            for r in range(nr):
                wmin, wmax = cover[r]
                if wmin < 0:
                    continue
                span = int(wmax - wmin + 1)
                assert span <= 4, f"rank spans {span} windows"
                rid = rank_id + r
                ranks_meta.append((rid, msg_off + r, int(wmin) & 3, span))
                for w in range(int(wmin), int(wmax) + 1):
                    win_ops[w].append((rid, w - int(wmin)))
            rank_id += nr
            msg_off += nr
        chunks.append({
            "calls": calls,
            "nranks": msg_off,
            "ranks": ranks_meta,
            "windows": [(w, win_ops[w]) for w in range(w0, w1)],
        })
    idx_arr = [np.concatenate(idx_cols[c], axis=1) if idx_cols[c]
               else np.zeros((P, 0), np.int16) for c in range(NCORES)]
    drel_arr = [np.concatenate(drel_cols[c], axis=1) if drel_cols[c]
                else np.zeros((P, 0), np.float32) for c in range(NCORES)]
    return idx_arr, drel_arr, chunks, rank_id


def _enc_idx(d):
    """Scatter token id for global dst d: partition/slot encoding whose
    parity-split SBUF accumulators reassemble densely into row 6272*o+dl."""
    o = d // NLOC
    dl = d % NLOC
    prel = dl // 196
    q = dl % 196
    t = q // 98
    g = q % 98
    s = 2 * g + t
    return (o // 4), 128 * s + 32 * (o % 4) + prel  # (half, idx)


def _build_scatter(src, dst):
    """L2 scatter-add schedule (src-partitioned, lo/hi dst halves).

    Returns per-core gather/scatter idx tables plus the shared round
    structure: rounds = [(phase, m_k, col16, ncol16)], Spad[2].
    """
    percore = []  # [c][ph] = (gidx_sorted, counts_sorted)
    Amats = []    # [c][ph] = slot-major [S, RCAP] edge-target idxs (TRASH pad)
    for c in range(NCORES):
        m = (src >= c * NLOC) & (src < (c + 1) * NLOC)
        s_all = src[m] - c * NLOC
        d = dst[m]
        half_all, idx_all = _enc_idx(d)
        ph_data = []
        ph_A = []
        for ph in (0, 1):
            sel = half_all == ph
            ss = s_all[sel]
            rr = idx_all[sel]
            o = np.argsort(ss, kind="stable")
            ss, rr = ss[o], rr[o]
            deg = np.bincount(ss, minlength=NLOC)
            nz = np.nonzero(deg)[0]
            gidx, lists = [], []
            pos = 0
            for i in nz:
                dg = int(deg[i])
                ci = -(-dg // RCAP)
                lst = rr[pos:pos + dg]
                for j in range(ci):
                    gidx.append(i)
                    lists.append(lst[j::ci])
                pos += dg
            counts = np.fromiter((len(x) for x in lists), np.int64, len(lists))
            order = np.argsort(-counts, kind="stable")
            gidx = np.asarray(gidx, np.int64)[order]
            counts = counts[order]
            A = np.full((len(lists), RCAP), TRASH, np.int64)
            for q, oi in enumerate(order):
                A[q, :counts[q]] = lists[oi]
            ph_data.append((gidx, counts))
            ph_A.append(A)
        percore.append(ph_data)
        Amats.append(ph_A)

    Spad = []
    for ph in (0, 1):
        smax = max(len(percore[c][ph][0]) for c in range(NCORES))
        Spad.append(-(-smax // P) * P)

    rounds = []
    col16 = 0
    for k in range(RCAP):
        for ph in (0, 1):
            m_k = max(int((percore[c][ph][1] > k).sum()) for c in range(NCORES))
            if m_k == 0:
                continue
            ncol = -(-m_k // 16)
            rounds.append((ph, m_k, col16, ncol))
            col16 += ncol

    g_arr = [[None, None] for _ in range(NCORES)]
    sc_arr = []
    for c in range(NCORES):
        for ph in (0, 1):
            gidx, counts = percore[c][ph]
            gp = np.full(Spad[ph], NPAD1 - 1, np.int64)  # pad -> all-zero p row
            gp[:len(gidx)] = gidx
            g_arr[c][ph] = _wrap_call(gp)
        cols = []
        for (ph, m_k, c16, ncol), k in zip(rounds, _round_ks(rounds)):
            A = Amats[c][ph]
            arr = np.full(ncol * 16, TRASH, np.int64)
            take = min(m_k, A.shape[0])
            arr[:take] = A[:take, k]
            arr[m_k:] = -1  # beyond num_idxs: trailing pad
            cols.append(_wrap_call(arr))
        sc_arr.append(np.concatenate(cols, axis=1) if cols
                      else np.zeros((P, 0), np.int16))
    return g_arr, sc_arr, rounds, Spad


def _round_ks(rounds):
    """Recover per-round k (edge position) from the interleaved round list."""
    kctr = {0: 0, 1: 0}
    ks = []
    for ph, m_k, c16, ncol in rounds:
        ks.append(kctr[ph])
        kctr[ph] += 1
    return ks


def _build_schedule(edge_index):
    src = np.asarray(edge_index[0], dtype=np.int64)
    dst = np.asarray(edge_index[1], dtype=np.int64)
    deg = np.bincount(dst, minlength=N).astype(np.float32)
    recip = 1.0 / np.maximum(deg, 1.0)

    # L1: dst-partitioned; window/slot from local node l = p*49 + k
    l1 = []
    for c in range(NCORES):
        m = (dst >= c * NLOC) & (dst < (c + 1) * NLOC)
        s, d = src[m], dst[m] - c * NLOC
        win = d % NW1
        slot = d // NW1
        blo = s < XSPLIT
        l1.append([
            (s[blo], win[blo], slot[blo]),
            (s[~blo] - XSPLIT, win[~blo], slot[~blo]),
        ])
    i1, d1, chunks1, R1 = _build_layer(
        l1, NW1, list(range(0, 45, L1_WCHUNK)) + [46, 48, NW1], 2)

    g_arr, sc_arr, rounds, Spad = _build_scatter(src, dst)
    return i1, d1, chunks1, R1, g_arr, sc_arr, rounds, Spad, recip


def kernel(x, edge_index, w1_l, b1, w1_r, w2_l, b2, w2_r):
    import concourse.bacc as bacc
    import concourse.mybir as mybir
    import concourse.tile as tile
    from concourse.bass_utils import run_bass_kernel_spmd
    from concourse.library_config import mlp
    from concourse.masks import make_identity

    x = np.asarray(x, np.float32)
    (i1, d1, chunks1, R1, g_arr, sc_arr, rounds, Spad,
     recip) = _build_schedule(np.asarray(edge_index))
    CRMAX = max(ch["nranks"] for ch in chunks1)

    xlo = x[:XSPLIT].astype(f16)
    xhi = x[XSPLIT:].astype(f16)
    iota_np = np.tile((np.arange(1024) % 512).astype(np.float32)[None, :],
                      (P, 1)).astype(f16)

    T16_1 = i1[0].shape[1]
    T16_S = sc_arr[0].shape[1]

    nc = bacc.Bacc("TRN2", dynamic_dma_scratch_size=49152)
    dt = mybir.dt
    t_xlo = nc.declare_dram_parameter("xlo", [XSPLIT, DIN], dt.float16, isOutput=False)
    t_xhi = nc.declare_dram_parameter("xhi", [N - XSPLIT, DIN], dt.float16, isOutput=False)
    t_i1 = nc.declare_dram_parameter("i1", [P, T16_1], dt.int16, isOutput=False)
    t_d1 = nc.declare_dram_parameter("d1", [P, R1], dt.float32, isOutput=False)
    t_gl = nc.declare_dram_parameter("gl", [P, Spad[0] // 16], dt.int16, isOutput=False)
    t_gh = nc.declare_dram_parameter("gh", [P, Spad[1] // 16], dt.int16, isOutput=False)
    t_sc = nc.declare_dram_parameter("sc", [P, T16_S], dt.int16, isOutput=False)
    t_xoT = nc.declare_dram_parameter("xoT", [DIN, NW1, P], dt.float16, isOutput=False)
    t_w1l = nc.declare_dram_parameter("w1l", [DIN, HID], dt.float16, isOutput=False)
    t_w1r = nc.declare_dram_parameter("w1r", [DIN, HID], dt.float16, isOutput=False)
    t_w2l = nc.declare_dram_parameter("w2l", [P, HID // P, OUT], dt.float16, isOutput=False)
    t_w2r = nc.declare_dram_parameter("w2r", [P, HID // P, OUT], dt.float16, isOutput=False)
    t_b1 = nc.declare_dram_parameter("b1", [P, HID // P], dt.float32, isOutput=False)
    t_b2 = nc.declare_dram_parameter("b2r", [P, OUT], dt.float32, isOutput=False)
    t_rc = nc.declare_dram_parameter("rc", [P, NW1], dt.float32, isOutput=False)
    t_rc40 = nc.declare_dram_parameter("rc40", [P, NW1, OUT], dt.float16, isOutput=False)
    t_iota = nc.declare_dram_parameter("iota", [P, 1024], dt.float16, isOutput=False)
    t_out = nc.declare_dram_parameter("out", [P, NW1, OUT], dt.float16, isOutput=True)

    t_p = nc.dram_tensor("ptab", [P, NW1, DIN], dt.float16)        # row l = p*49+k
    t_cont = nc.dram_tensor("cont", [NCORES * NPAD1, OUT], dt.float8e4)
    t_rs = nc.dram_tensor("rsout", [NPAD1, OUT], dt.float8e4)

    AluOp = mybir.AluOpType
    AF = mybir.ActivationFunctionType

    with tile.TileContext(nc) as tc:
        with tc.tile_pool(name="const", bufs=1) as cpool, \
             tc.tile_pool(name="epi", bufs=1) as epool:
            nc.gpsimd.load_library(mlp)
            ident = cpool.tile([P, P], dt.float16)
            make_identity(nc, ident[:])
            # i1 first so gather-0 starts early; big/late tables (xoT, sc,
            # gl, gh, rc40) queue after it.
            i1_t = cpool.tile([P, T16_1], dt.int16)
            nc.sync.dma_start(i1_t[:], t_i1[:])
            d1_t = cpool.tile([P, R1], dt.float32)
            nc.sync.dma_start(d1_t[:], t_d1[:])
            iota_t = cpool.tile([P, 1024], dt.float16)
            nc.sync.dma_start(iota_t[:], t_iota[:])
            w1l_t = cpool.tile([DIN, HID], dt.float16)
            nc.sync.dma_start(w1l_t[:], t_w1l[:])
            w1r_t = cpool.tile([DIN, HID], dt.float16)
            nc.sync.dma_start(w1r_t[:], t_w1r[:])
            w2l_t = cpool.tile([P, HID // P, OUT], dt.float16)
            nc.sync.dma_start(w2l_t[:], t_w2l[:])
            w2r_t = cpool.tile([P, HID // P, OUT], dt.float16)
            nc.sync.dma_start(w2r_t[:], t_w2r[:])
            b1_t = cpool.tile([P, HID // P], dt.float32)
            nc.sync.dma_start(b1_t[:], t_b1[:])
            b2_t = cpool.tile([P, OUT], dt.float32)
            nc.sync.dma_start(b2_t[:], t_b2[:])
            rc_t = cpool.tile([P, NW1], dt.float32)
            nc.sync.dma_start(rc_t[:], t_rc[:])
            # late tables: tiles allocated here, loads issued from Pool after
            # the first gather preps so they don't hog the DMA queue early
            xoT_t = cpool.tile([DIN, NW1, P], dt.float16)
            sc_t = cpool.tile([P, T16_S], dt.int16)
            gl_t = cpool.tile([P, Spad[0] // 16], dt.int16)
            gh_t = cpool.tile([P, Spad[1] // 16], dt.int16)
            rc40_t = cpool.tile([P, NW1, OUT], dt.float16)
            qbuf = cpool.tile([P, NW1, OUT], dt.float16)
            warm = cpool.tile([P, 2], dt.float32)
            nc.scalar.activation(warm[:, 0:1], b2_t[:, 0:1], AF.Exp)
            nc.scalar.activation(warm[:, 1:2], b2_t[:, 1:2], AF.Ln)
            ptA = cpool.tile([P, PTK, DIN], dt.float16)
            ptB = cpool.tile([P, PTK, DIN], dt.float16)
            nc.vector.memset(ptA[:], 0.0)
            nc.vector.memset(ptB[:], 0.0)

            # ---------------- Layer 1 ----------------
            with tc.tile_pool(name="msg", bufs=3) as mpool, \
                 tc.tile_pool(name="sm", bufs=16) as spool, \
                 tc.tile_pool(name="work", bufs=3) as wpool, \
                 tc.tile_pool(name="psumA", bufs=2, space="PSUM") as ppool, \
                 tc.tile_pool(name="psumB", bufs=2, space="PSUM") as ppoolb, \
                 tc.tile_pool(name="psumC", bufs=1, space="PSUM") as ppoolc:
                for _mb in range(3):
                    mz = mpool.tile([P, CRMAX, DIN], dt.float16, tag="msg")
                    nc.vector.memset(mz[:], 0.0)
                pt_tiles = [ptA, ptB]
                pt_pend = 0          # windows staged in current pt tile
                pt_w0 = 0
                pt_i = 0
                for ci, ch in enumerate(chunks1):
                    cr = ch["nranks"]
                    if cr == 0:
                        continue
                    msg = mpool.tile([P, CRMAX, DIN], dt.float16, tag="msg")
                    off = 0
                    for b, nidx, used, cum16 in ch["calls"]:
                        tbl = t_xlo[:] if b == 0 else t_xhi[:]
                        nc.gpsimd.dma_gather(
                            msg[:, off:off + (-(-used // P)), :], tbl,
                            i1_t[:, cum16:cum16 + used // 16],
                            used, used, DIN, single_packet=False)
                        off += nidx // P
                    if ci == 0:
                        nc.gpsimd.dma_start(xoT_t[:], t_xoT[:])
                    elif ci == 1:
                        nc.gpsimd.dma_start(sc_t[:], t_sc[:])
                    elif ci == 2:
                        nc.gpsimd.dma_start(gl_t[:], t_gl[:])
                        nc.gpsimd.dma_start(gh_t[:], t_gh[:])
                        nc.gpsimd.dma_start(rc40_t[:], t_rc40[:])
                    rank_info = {rid: (lr, q0, span) for rid, lr, q0, span in ch["ranks"]}
                    S_tiles = {}
                    for w, ops in ch["windows"]:
                        for rid, blk in ops:
                            if rid not in S_tiles:
                                lr, q0, span = rank_info[rid]
                                S = spool.tile([P, 512], dt.float16, tag="S1")
                                nc.vector.tensor_scalar(
                                    out=S[:, :span * P],
                                    in0=iota_t[:, q0 * P:(q0 + span) * P],
                                    scalar1=d1_t[:, rid:rid + 1], scalar2=None,
                                    op0=AluOp.is_equal)
                                S_tiles[rid] = S
                        pagg = ppool.tile([P, P], dt.float32, tag="pagg")
                        if not ops:
                            nc.vector.memset(pagg[:], 0.0)
                        for j, (rid, blk) in enumerate(ops):
                            lr = rank_info[rid][0]
                            nc.tensor.matmul(
                                pagg[:], lhsT=S_tiles[rid][:, blk * P:(blk + 1) * P],
                                rhs=msg[:, lr, :],
                                start=(j == 0), stop=(j == len(ops) - 1))
                        am = wpool.tile([P, P], dt.float16, tag="am")
                        nc.scalar.activation(am[:], pagg[:], AF.Copy,
                                             scale=rc_t[:, w:w + 1])
                        pamT = ppoolc.tile([P, P], dt.float16, tag="pamT")
                        nc.tensor.transpose(out=pamT[:], in_=am[:], identity=ident[:])
                        amT = wpool.tile([P, P], dt.float16, tag="amT")
                        nc.scalar.activation(amT[:], pamT[:], AF.Copy)
                        pq = ppoolc.tile([P, OUT], dt.float32, tag="pq")
                        qq = ppoolc.tile([P, OUT], dt.float32, tag="qq")
                        for bjj in range(HID // P):
                            ph_ = ppoolb.tile([P, P], dt.float32, tag="ph")
                            nc.tensor.matmul(ph_[:], lhsT=w1l_t[:, bjj * P:(bjj + 1) * P],
                                             rhs=amT[:], start=True, stop=False)
                            nc.tensor.matmul(ph_[:], lhsT=w1r_t[:, bjj * P:(bjj + 1) * P],
                                             rhs=xoT_t[:, w, :], start=False, stop=True)
                            hT = wpool.tile([P, P], dt.float16, tag="hT")
                            nc.scalar.activation(hT[:], ph_[:], AF.Relu,
                                                 bias=b1_t[:, bjj:bjj + 1])
                            nc.tensor.matmul(pq[:], lhsT=hT[:], rhs=w2l_t[:, bjj, :],
                                             start=(bjj == 0), stop=(bjj == 3))
                            nc.tensor.matmul(qq[:], lhsT=hT[:], rhs=w2r_t[:, bjj, :],
                                             start=(bjj == 0), stop=(bjj == 3))
                        nc.scalar.activation(qbuf[:, w, :], qq[:], AF.Copy)
                        pt = pt_tiles[pt_i]
                        nc.scalar.activation(pt[:, pt_pend, :OUT], pq[:], AF.Copy)
                        pt_pend += 1
                        if (pt_pend == PTK or w == NW1 - 1
                                or (w >= 31 and pt_pend >= 8)):
                            nc.sync.dma_start(t_p[:, pt_w0:pt_w0 + pt_pend, :],
                                              pt[:, :pt_pend, :])
                            pt_w0 += pt_pend
                            pt_pend = 0
                            pt_i ^= 1

            # fold b2 into qbuf early (overlaps L2 phase / RS wait)
            nc.vector.tensor_tensor(
                out=qbuf[:], in0=qbuf[:],
                in1=b2_t[:, None, :].to_broadcast([P, NW1, OUT]), op=AluOp.add)

            # ---------------- Layer 2: SBUF parity-split scatter-add ----------
            with tc.tile_pool(name="scat", bufs=1) as zpool:
                # accumulators: [half][parity] -> [128, 99, 40] (group 98 = trash)
                acc00 = zpool.tile([P, 99, OUT], dt.float16, tag="acc00")
                acc01 = zpool.tile([P, 99, OUT], dt.float16, tag="acc01")
                acc10 = zpool.tile([P, 99, OUT], dt.float16, tag="acc10")
                acc11 = zpool.tile([P, 99, OUT], dt.float16, tag="acc11")
                accs = [[acc00, acc01], [acc10, acc11]]
                for h in (0, 1):
                    for t in (0, 1):
                        nc.vector.memset(accs[h][t][:], 0.0)
                t_p_flat = t_p[:].rearrange("p k j -> (p k) j")
                praw_l = zpool.tile([P, Spad[0] // P, DIN], dt.float16)
                nc.gpsimd.dma_gather(
                    praw_l[:], t_p_flat, gl_t[:],
                    Spad[0], Spad[0], DIN, single_packet=False)
                p_lo = zpool.tile([P, Spad[0] // P, OUT], dt.float16)
                nc.scalar.activation(p_lo[:], praw_l[:, :, :OUT], AF.Copy)
                praw_h = zpool.tile([P, Spad[1] // P, DIN], dt.float16)
                nc.gpsimd.dma_gather(
                    praw_h[:], t_p_flat, gh_t[:],
                    Spad[1], Spad[1], DIN, single_packet=False)
                p_hi = zpool.tile([P, Spad[1] // P, OUT], dt.float16)
                nc.scalar.activation(p_hi[:], praw_h[:, :, :OUT], AF.Copy)

                for ph, m_k, c16, ncol in rounds:
                    ptile = p_lo if ph == 0 else p_hi
                    nr = -(-m_k // P)
                    nc.gpsimd.dma_scatter_add(
                        accs[ph][0][:], ptile[:, :nr, :],
                        sc_t[:, c16:c16 + ncol],
                        m_k, m_k, OUT,
                        single_packet=False,
                        sbuf_tokens_per_rank=P,
                        parity_reg=0,
                        out_ap_other=accs[ph][1][:])

                # dense write-out: row 25088*h + 196*p + 98*t + g == 6272*o + dl
                tc_v = t_cont[:].rearrange("(h p q) j -> h p q j", h=2, p=P)
                for h in (0, 1):
                    for t in (0, 1):
                        acc8 = zpool.tile([P, 98, OUT], dt.float8e4,
                                          tag=f"acc8_{h}{t}")
                        nc.scalar.activation(acc8[:], accs[h][t][:, :98, :],
                                             AF.Copy)
                        nc.sync.dma_start(tc_v[h][:, 98 * t:98 * (t + 1), :],
                                          acc8[:])

                # ---------------- ReduceScatter + epilogue ----------------
                nc.gpsimd.collective_compute(
                    "ReduceScatter", AluOp.add, replica_groups=[list(range(NCORES))],
                    ins=[t_cont[:]], outs=[t_rs[:]])

                rsb = epool.tile([P, NW1, OUT], dt.float8e4, tag="rsb")
                nc.sync.dma_start(
                    rsb[:], t_rs[:].rearrange("(p k) j -> p k j", p=P))
                HALVES = [(0, 13), (13, 25), (25, 37), (37, NW1)]
                zts, ezs, ssums, lsums = [], [], [], []
                for h, (k0, k1) in enumerate(HALVES):
                    KH = k1 - k0
                    zt = epool.tile([P, KH, OUT], dt.float16, tag=f"zt{h}")
                    nc.vector.tensor_tensor(out=zt[:], in0=rsb[:, k0:k1, :],
                                            in1=rc40_t[:, k0:k1, :], op=AluOp.mult)
                    nc.vector.tensor_tensor(out=zt[:], in0=zt[:],
                                            in1=qbuf[:, k0:k1, :], op=AluOp.add)
                    zts.append(zt)
                    ez = epool.tile([P, KH, OUT], dt.float32, tag=f"ez{h}")
                    nc.scalar.activation(ez[:], zt[:], AF.Exp)
                    ezs.append(ez)
                for h, (k0, k1) in enumerate(HALVES):
                    KH = k1 - k0
                    ssum = epool.tile([P, KH, 1], dt.float32, tag=f"ssum{h}")
                    nc.vector.tensor_reduce(ssum[:], ezs[h][:],
                                            axis=mybir.AxisListType.X, op=AluOp.add)
                    ssums.append(ssum)
                for h, (k0, k1) in enumerate(HALVES):
                    KH = k1 - k0
                    lsum = epool.tile([P, KH, 1], dt.float32, tag=f"lsum{h}")
                    nc.scalar.activation(lsum[:], ssums[h][:], AF.Ln)
                    lsums.append(lsum)
                for h, (k0, k1) in enumerate(HALVES):
                    KH = k1 - k0
                    ot = epool.tile([P, KH, OUT], dt.float16, tag=f"ot{h}")
                    nc.vector.tensor_tensor(
                        out=ot[:], in0=zts[h][:],
                        in1=lsums[h][:].to_broadcast([P, KH, OUT]), op=AluOp.subtract)
                    nc.sync.dma_start(t_out[:, k0:k1, :], ot[:])

    nc.compile()

    b2b_np = np.tile(np.asarray(b2, np.float32)[None, :], (P, 1))
    in_maps = []
    for c in range(NCORES):
        xl = np.zeros((NPAD1, DIN), np.float32)
        xl[:NLOC] = x[c * NLOC:(c + 1) * NLOC]
        xoT = np.ascontiguousarray(
            xl.reshape(P, NW1, DIN).transpose(2, 1, 0)).astype(f16)
        rcf = np.ones(NPAD1, np.float32)
        rcf[:NLOC] = recip[c * NLOC:(c + 1) * NLOC]
        rcc = rcf.reshape(P, NW1).copy()
        in_maps.append({
            "xlo": xlo, "xhi": xhi,
            "i1": i1[c], "d1": d1[c],
            "gl": g_arr[c][0], "gh": g_arr[c][1], "sc": sc_arr[c],
            "xoT": xoT, "rc": rcc, "iota": iota_np,
            "rc40": np.ascontiguousarray(np.broadcast_to(rcc[:, :, None], (P, NW1, OUT))).astype(f16),
            "w1l": np.asarray(w1_l).astype(f16),
            "w1r": np.asarray(w1_r).astype(f16),
            "w2l": np.ascontiguousarray(
                np.asarray(w2_l).astype(f16).reshape(HID // P, P, OUT)
                .transpose(1, 0, 2)),
            "w2r": np.ascontiguousarray(
                np.asarray(w2_r).astype(f16).reshape(HID // P, P, OUT)
                .transpose(1, 0, 2)),
            "b1": np.asarray(b1, np.float32).reshape(HID // P, P).T.copy(),
            "b2r": b2b_np,
        })
    res = run_bass_kernel_spmd(nc, in_maps, list(range(NCORES)))
    out = np.concatenate(
        [res.results[c]["out"].reshape(NPAD1, OUT)[:NLOC] for c in range(NCORES)],
        axis=0)
    kernel.last_results = res
    kernel.last_nc = nc
    return out.astype(np.float32)
